# revision 1
# baseline (speedup 1.0000x reference)
"""EquivariantEdgeConv fused Bass kernel for one TRN2 chip (8 NeuronCores).

Strategy (node-sharded scatter, edge-bucketed message passing):
  - Nodes are sharded: core c owns nodes [1024c, 1024c+1024), i.e. 8
    buckets of 128 nodes each. Each core receives exactly the edges whose
    *destination* lands in its node range, grouped by 128-node bucket and
    padded per-bucket to a multiple of 128 (shared static capacity).
  - Per 128-edge tile, on device:
      * indirect-gather x[src] (+pos[src]) rows and pos[dst] rows from
        replicated DRAM copies,
      * edge geometry (vec, len, Y1) on DVE/ACT,
      * hT = silu(len * w1) built transposed via a PE transpose + a rank-1
        matmul, then w = hT.T @ w2p streamed through PSUM in two
        2048-column halves (column-permuted + path-normalized w2),
      * the four tensor-product paths as elementwise products (DVE,
        reading w straight from PSUM) + segmented reduces,
      * scatter-add via matmul with an on-device one-hot of the local
        destination index: outT += m.T @ onehot, accumulated in SBUF.
  - Per bucket, the gated output block (o3.Linear + silu/sigmoid gate) runs
    transposed on PE/ACT, is transposed back and DMA'd to the core's
    output slice. Outputs concatenate across cores - no collective needed.

The kernel is self-contained: shapes/sharding are hardcoded for
N=8192 nodes, E=65536 edges, irreps 48x0e + 16x1o, H=64.
"""

import sys

if "/opt/trn_rl_repo" not in sys.path:
    sys.path.insert(0, "/opt/trn_rl_repo")

import numpy as np

import concourse.bacc as bacc
import concourse.bass as bass
import concourse.mybir as mybir
import concourse.tile as tile
from concourse.bass import AP
from concourse.bass_utils import run_bass_kernel_spmd

M0, M1, H = 48, 16, 64
N_NODES, N_EDGES, N_CORES = 8192, 65536, 8
NODES_PER_CORE = N_NODES // N_CORES          # 1024
BUCKETS = NODES_PER_CORE // 128              # 8 buckets of 128 nodes per core
FP = mybir.dt.float32
BF = mybir.dt.bfloat16
I32 = mybir.dt.int32

# path normalizations (cA..cD) and the radial-MLP 1/sqrt(H), folded into w2
CA = 1.0 / np.sqrt(M0 * 2.0)
CB = 1.0 / np.sqrt(3.0 * M1 * 2.0)
CC = 1.0 / np.sqrt(M0 * 2.0)
CD = 1.0 / np.sqrt(M1 * 2.0)
SQRT3 = float(np.sqrt(3.0))

# per-half column layout of the permuted w2 (see _permute_w2):
#   [A(o:24x48) | B(o:24x16) | C(o:8x48) | D(o:8x16)] = 1152+384+384+128 = 2048
A_OFF, B_OFF, C_OFF, D_OFF = 0, 1152, 1536, 1920


def _permute_w2(w2: np.ndarray) -> np.ndarray:
    """Permute + scale w2 columns into the on-device layout.

    Original column order (from reference reshape):
      A: (i,o) i-major, i,o in 48      cols [0, 2304)
      B: (i,o) i in 16, o in 48        cols [2304, 3072)
      C: (i,o) i in 48, o in 16        cols [3072, 3840)
      D: (i,o) i,o in 16               cols [3840, 4096)
    Target: two 2048-col halves h=0,1; within a half:
      A rows o in [24h,24h+24) laid (o,i) o-major  -> 1152 cols
      B rows o in [24h,24h+24) laid (o,i)          -> 384
      C rows o in [8h,8h+8)    laid (o,i)          -> 384
      D rows o in [8h,8h+8)    laid (o,i)          -> 128
    """
    perm = np.empty(4096, np.int64)
    scale = np.empty(4096, np.float32)
    inv_sqrt_h = 1.0 / np.sqrt(H)
    for h in (0, 1):
        base = 2048 * h
        # A
        for oo in range(24):
            o = 24 * h + oo
            for i in range(48):
                perm[base + A_OFF + oo * 48 + i] = i * 48 + o
                scale[base + A_OFF + oo * 48 + i] = CA * inv_sqrt_h
        # B
        for oo in range(24):
            o = 24 * h + oo
            for i in range(16):
                perm[base + B_OFF + oo * 16 + i] = 2304 + i * 48 + o
                scale[base + B_OFF + oo * 16 + i] = CB * inv_sqrt_h
        # C
        for oo in range(8):
            o = 8 * h + oo
            for i in range(48):
                perm[base + C_OFF + oo * 48 + i] = 3072 + i * 16 + o
                scale[base + C_OFF + oo * 48 + i] = CC * inv_sqrt_h
        # D
        for oo in range(8):
            o = 8 * h + oo
            for i in range(16):
                perm[base + D_OFF + oo * 16 + i] = 3840 + i * 16 + o
                scale[base + D_OFF + oo * 16 + i] = CD * inv_sqrt_h
    return (w2[:, perm] * scale[None, :]).astype(np.float32)


def _wns_block(wns: np.ndarray) -> np.ndarray:
    """[48,48] lhsT for the 1o o3.Linear on (o,m)-interleaved rows:
    lhsT[(i,m),(o,m')] = Wns[i,o] * delta(m,m') / sqrt(M1)."""
    out = np.zeros((48, 48), np.float32)
    for i in range(16):
        for m in range(3):
            for o in range(16):
                out[i * 3 + m, o * 3 + m] = wns[i, o] / np.sqrt(M1)
    return out


def _prep_edges(edge_index: np.ndarray):
    """Bucket/pad edges by destination. Returns per-core index arrays and
    the shared per-bucket tile count."""
    src, dst = edge_index[0].astype(np.int64), edge_index[1].astype(np.int64)
    gb = dst >> 7  # global bucket 0..63
    order = np.argsort(gb, kind="stable")
    src_s, dst_s, gb_s = src[order], dst[order], gb[order]
    counts = np.bincount(gb_s, minlength=64)
    cap = int(np.ceil(counts.max() / 128) * 128)
    tiles_per_bucket = cap // 128

    srcidx = np.zeros((N_CORES, BUCKETS * cap), np.int32)
    dstpos = np.zeros((N_CORES, BUCKETS * cap), np.int32)
    dstloc = np.full((N_CORES, BUCKETS * cap), 300.0, np.float32)
    starts = np.concatenate([[0], np.cumsum(counts)])
    for g in range(64):
        c, b = g >> 3, g & 7
        s, e = starts[g], starts[g + 1]
        n = e - s
        o = b * cap
        srcidx[c, o : o + n] = src_s[s:e]
        dstpos[c, o : o + n] = dst_s[s:e]
        dstloc[c, o : o + n] = (dst_s[s:e] - (g << 7)).astype(np.float32)
    # reshape to [BUCKETS*128, T]: column t = tile t's per-partition indices
    def to_cols(a):
        out = np.empty((N_CORES, BUCKETS * 128, tiles_per_bucket), a.dtype)
        for b in range(BUCKETS):
            blk = a[:, b * cap : (b + 1) * cap].reshape(N_CORES, tiles_per_bucket, 128)
            out[:, b * 128 : (b + 1) * 128, :] = blk.transpose(0, 2, 1)
        return out
    return to_cols(srcidx), to_cols(dstpos), to_cols(dstloc), tiles_per_bucket


def build_kernel(tiles_per_bucket: int, reps: int = 1) -> bass.Bass:
    nc = bacc.Bacc(None, target_bir_lowering=False, debug=False)
    d_xcat = nc.declare_dram_parameter("xcat", [N_NODES, 100], FP, isOutput=False)
    d_posp = nc.declare_dram_parameter("posp", [N_NODES, 4], FP, isOutput=False)
    T = tiles_per_bucket
    d_srcidx = nc.declare_dram_parameter("srcidx", [BUCKETS * 128, T], I32, isOutput=False)
    d_dstpos = nc.declare_dram_parameter("dstpos", [BUCKETS * 128, T], I32, isOutput=False)
    d_dstloc = nc.declare_dram_parameter("dstloc", [BUCKETS * 128, T], FP, isOutput=False)
    d_w1 = nc.declare_dram_parameter("w1", [1, H], BF, isOutput=False)
    d_w2p = nc.declare_dram_parameter("w2p", [H, 4096], BF, isOutput=False)
    d_ws = nc.declare_dram_parameter("ws", [M0, M0], FP, isOutput=False)
    d_wg = nc.declare_dram_parameter("wg", [M0, M0], FP, isOutput=False)
    d_wns = nc.declare_dram_parameter("wns", [48, 48], FP, isOutput=False)
    d_ident = nc.declare_dram_parameter("ident", [128, 128], FP, isOutput=False)
    d_iota = nc.declare_dram_parameter("iota", [128, 128], BF, isOutput=False)
    d_out = nc.declare_dram_parameter("out", [NODES_PER_CORE, M0], FP, isOutput=True)

    with tile.TileContext(nc) as tc, tc.tile_pool(name="consts", bufs=1) as cp:
        w1_sb = cp.tile([1, H], BF)
        w2p_sb = cp.tile([H, 4096], BF)
        ws_sb = cp.tile([M0, M0], FP)
        wg_sb = cp.tile([M0, M0], FP)
        wns_sb = cp.tile([48, 48], FP)
        ident_sb = cp.tile([128, 128], FP)
        iota_sb = cp.tile([128, 128], BF)
        for sb, dr in (
            (w1_sb, d_w1), (w2p_sb, d_w2p), (ws_sb, d_ws), (wg_sb, d_wg),
            (wns_sb, d_wns), (ident_sb, d_ident), (iota_sb, d_iota),
        ):
            nc.sync.dma_start(out=sb[:], in_=dr[:])

        with (
            tc.tile_pool(name="idx", bufs=2) as idxp,
            tc.tile_pool(name="gath", bufs=4) as gathp,
            tc.tile_pool(name="geo", bufs=4) as geop,
            tc.tile_pool(name="work", bufs=3) as workp,
            tc.tile_pool(name="msg", bufs=3) as msgp,
            tc.tile_pool(name="accs", bufs=1) as accp,
            tc.tile_pool(name="accps", bufs=1, space="PSUM") as accpp,
            tc.tile_pool(name="wps", bufs=1, space="PSUM") as wpsp,
            tc.tile_pool(name="ps_small", bufs=2, space="PSUM") as psp,
        ):
          rep_ctx = tc.For_i(0, reps, 1) if reps > 1 else None
          if rep_ctx is not None:
              rep_ctx.__enter__()
          if True:
            for b in range(BUCKETS):
                sc_s = accpp.tile([48, 128], FP, tag="acc_s")
                sc_v = accpp.tile([48, 128], FP, tag="acc_v")
                bidx_s = idxp.tile([128, T], I32, tag="idx_s")
                bidx_d = idxp.tile([128, T], I32, tag="idx_d")
                bdl = idxp.tile([128, T], FP, tag="dl")
                nc.sync.dma_start(out=bidx_s[:], in_=d_srcidx[128 * b : 128 * (b + 1), :])
                nc.sync.dma_start(out=bidx_d[:], in_=d_dstpos[128 * b : 128 * (b + 1), :])
                nc.sync.dma_start(out=bdl[:], in_=d_dstloc[128 * b : 128 * (b + 1), :])
                for t in range(tiles_per_bucket):
                    idx_s = bidx_s[:, t : t + 1]
                    idx_d = bidx_d[:, t : t + 1]
                    dl = bdl[:, t : t + 1]
                    xg = gathp.tile([128, 100], FP, tag="xg")
                    pd = gathp.tile([128, 4], FP, tag="pd")
                    nc.gpsimd.indirect_dma_start(
                        out=xg[:], out_offset=None, in_=d_xcat[:],
                        in_offset=bass.IndirectOffsetOnAxis(ap=idx_s[:, :1], axis=0),
                    )
                    nc.gpsimd.indirect_dma_start(
                        out=pd[:], out_offset=None, in_=d_posp[:],
                        in_offset=bass.IndirectOffsetOnAxis(ap=idx_d[:, :1], axis=0),
                    )
                    # ---- geometry ----
                    geo = geop.tile([128, 4], FP, tag="geo")   # [vec(3) | len]
                    sq = geop.tile([128, 3], FP, tag="sq")
                    lensq = geop.tile([128, 1], FP, tag="lensq")
                    invl = geop.tile([128, 1], FP, tag="invl")
                    y1 = geop.tile([128, 3], FP, tag="y1")
                    vec = geo[:, 0:3]
                    nc.vector.tensor_tensor(
                        out=vec, in0=pd[:, 0:3], in1=xg[:, 96:99],
                        op=mybir.AluOpType.subtract,
                    )
                    nc.vector.tensor_tensor(
                        out=sq[:], in0=vec, in1=vec, op=mybir.AluOpType.mult
                    )
                    nc.vector.reduce_sum(
                        lensq[:], sq[:], axis=mybir.AxisListType.X
                    )
                    nc.scalar.activation(
                        geo[:, 3:4], lensq[:], mybir.ActivationFunctionType.Sqrt
                    )
                    nc.vector.tensor_scalar_max(geo[:, 3:4], geo[:, 3:4], 1e-8)
                    nc.vector.reciprocal(invl[:], geo[:, 3:4])
                    nc.vector.tensor_scalar_mul(invl[:], invl[:], SQRT3)
                    nc.vector.tensor_scalar_mul(y1[:], vec, invl[:, :1])
                    # ---- hT = silu(len * w1), built transposed ----
                    # len row via matmul: len_col.T @ I = [1,128]
                    lenT_ps = psp.tile([1, 128], FP, tag="ps")
                    nc.tensor.matmul(
                        lenT_ps[:], lhsT=geo[:, 3:4], rhs=ident_sb[:],
                        start=True, stop=True,
                    )
                    lenrow = geop.tile([1, 128], BF, tag="lenrow")
                    nc.scalar.activation(
                        lenrow[:], lenT_ps[0:1, :],
                        mybir.ActivationFunctionType.Copy,
                    )
                    hpre_ps = psp.tile([H, 128], FP, tag="ps")
                    nc.tensor.matmul(
                        hpre_ps[:], lhsT=w1_sb[:1, :], rhs=lenrow[:1, :],
                        start=True, stop=True,
                    )
                    hsig = geop.tile([H, 128], FP, tag="hsig")
                    hT = geop.tile([H, 128], BF, tag="hT")
                    nc.scalar.activation(
                        hsig[:], hpre_ps[:], mybir.ActivationFunctionType.Sigmoid
                    )
                    nc.vector.tensor_tensor(
                        out=hT[:], in0=hpre_ps[:], in1=hsig[:],
                        op=mybir.AluOpType.mult,
                    )
                    # ---- xvy[e,i] = sum_m xv[e,i,m] * Y1[e,m] ----
                    xvy = geop.tile([128, 16], FP, tag="xvy")
                    pvy = geop.tile([128, 48], FP, tag="pvy")
                    xv3 = xg[:, 48:96].rearrange("p (i m) -> p i m", m=3)
                    y1b16 = y1[:].rearrange("p (o m) -> p o m", o=1).to_broadcast(
                        [128, 16, 3]
                    )
                    nc.vector.tensor_tensor(
                        out=pvy[:].rearrange("p (i m) -> p i m", m=3),
                        in0=xv3, in1=y1b16, op=mybir.AluOpType.mult,
                    )
                    nc.vector.reduce_sum(
                        xvy[:], pvy[:].rearrange("p (i m) -> p i m", m=3),
                        axis=mybir.AxisListType.X,
                    )
                    # ---- per-edge TP, two 2048-col halves ----
                    msA = msgp.tile([128, 48], FP, tag="msA")
                    msB = msgp.tile([128, 48], FP, tag="msB")
                    zC = msgp.tile([128, 16], FP, tag="zC")
                    mvD = msgp.tile([128, 48], FP, tag="mvD")
                    m_t = msgp.tile([128, 96], BF, tag="m")
                    xs = xg[:, 0:48]
                    for hh in (0, 1):
                        wps = wpsp.tile([128, 2048], FP, tag="wps")
                        for j in range(4):
                            nc.tensor.matmul(
                                wps[:, 512 * j : 512 * (j + 1)],
                                lhsT=hT[:, :],
                                rhs=w2p_sb[:, 2048 * hh + 512 * j : 2048 * hh + 512 * (j + 1)],
                                start=True, stop=True,
                            )
                        prodA = workp.tile([128, 1152], FP, tag="prodA")
                        prodB = workp.tile([128, 384], FP, tag="prodB")
                        prodC = workp.tile([128, 384], FP, tag="prodC")
                        prodD = workp.tile([128, 384], FP, tag="prodD")
                        # A: sum_i xs[e,i] * wA[e,(o,i)]
                        nc.vector.tensor_tensor(
                            out=prodA[:].rearrange("p (o i) -> p o i", i=48),
                            in0=wps[:, A_OFF : A_OFF + 1152].rearrange(
                                "p (o i) -> p o i", i=48
                            ),
                            in1=xs.rearrange("p (o i) -> p o i", o=1).to_broadcast(
                                [128, 24, 48]
                            ),
                            op=mybir.AluOpType.mult,
                        )
                        nc.vector.reduce_sum(
                            msA[:, 24 * hh : 24 * hh + 24],
                            prodA[:].rearrange("p (o i) -> p o i", i=48),
                            axis=mybir.AxisListType.X,
                        )
                        # B: sum_i xvy[e,i] * wB[e,(o,i)]
                        nc.vector.tensor_tensor(
                            out=prodB[:].rearrange("p (o i) -> p o i", i=16),
                            in0=wps[:, B_OFF : B_OFF + 384].rearrange(
                                "p (o i) -> p o i", i=16
                            ),
                            in1=xvy[:].rearrange("p (o i) -> p o i", o=1).to_broadcast(
                                [128, 24, 16]
                            ),
                            op=mybir.AluOpType.mult,
                        )
                        nc.vector.reduce_sum(
                            msB[:, 24 * hh : 24 * hh + 24],
                            prodB[:].rearrange("p (o i) -> p o i", i=16),
                            axis=mybir.AxisListType.X,
                        )
                        # C: z_C[e,o] = sum_i xs[e,i] * wC[e,(o,i)]
                        nc.vector.tensor_tensor(
                            out=prodC[:].rearrange("p (o i) -> p o i", i=48),
                            in0=wps[:, C_OFF : C_OFF + 384].rearrange(
                                "p (o i) -> p o i", i=48
                            ),
                            in1=xs.rearrange("p (o i) -> p o i", o=1).to_broadcast(
                                [128, 8, 48]
                            ),
                            op=mybir.AluOpType.mult,
                        )
                        nc.vector.reduce_sum(
                            zC[:, 8 * hh : 8 * hh + 8],
                            prodC[:].rearrange("p (o i) -> p o i", i=48),
                            axis=mybir.AxisListType.X,
                        )
                        # D: mv_D[e,(o,m)] = sum_i xv[e,(i,m)] * wD[e,(o,i)]
                        wD = wps[:, D_OFF : D_OFF + 128]
                        wD_omi = AP(wD.tensor, wD.offset,
                                    [wD.ap[0], [16, 8], [0, 3], [1, 16]])
                        xv = xg[:, 48:96]
                        xv_omi = AP(xv.tensor, xv.offset,
                                    [xv.ap[0], [0, 8], [1, 3], [3, 16]])
                        nc.vector.tensor_tensor(
                            out=prodD[:].rearrange("p (o m i) -> p o m i", m=3, i=16),
                            in0=wD_omi, in1=xv_omi, op=mybir.AluOpType.mult,
                        )
                        nc.vector.reduce_sum(
                            mvD[:, 24 * hh : 24 * hh + 24].rearrange(
                                "p (o m) -> p o m", m=3
                            ),
                            prodD[:].rearrange("p (o m i) -> p o m i", m=3, i=16),
                            axis=mybir.AxisListType.X,
                        )
                    # combine: ms = A + B ; mv = zC x Y1 + mvD
                    nc.vector.tensor_tensor(
                        out=m_t[:, 0:48], in0=msA[:], in1=msB[:],
                        op=mybir.AluOpType.add,
                    )
                    mvC = msgp.tile([128, 48], FP, tag="mvC")
                    nc.vector.tensor_tensor(
                        out=mvC[:].rearrange("p (o m) -> p o m", m=3),
                        in0=zC[:].rearrange("p (o m) -> p o m", m=1).to_broadcast(
                            [128, 16, 3]
                        ),
                        in1=y1[:].rearrange("p (o m) -> p o m", o=1).to_broadcast(
                            [128, 16, 3]
                        ),
                        op=mybir.AluOpType.mult,
                    )
                    nc.vector.tensor_tensor(
                        out=m_t[:, 48:96], in0=mvC[:], in1=mvD[:],
                        op=mybir.AluOpType.add,
                    )
                    # ---- scatter via one-hot matmuls, accumulate in SBUF ----
                    oh = msgp.tile([128, 128], BF, tag="oh")
                    nc.vector.tensor_scalar(
                        out=oh[:], in0=iota_sb[:], scalar1=dl[:, :1],
                        scalar2=None, op0=mybir.AluOpType.is_equal,
                    )
                    nc.tensor.matmul(
                        sc_s[:], lhsT=m_t[:, 0:48], rhs=oh[:],
                        start=(t == 0), stop=(t == tiles_per_bucket - 1),
                    )
                    nc.tensor.matmul(
                        sc_v[:], lhsT=m_t[:, 48:96], rhs=oh[:],
                        start=(t == 0), stop=(t == tiles_per_bucket - 1),
                    )
                # ---- node stage for bucket b (all transposed [feat, node]) ----
                acc_s = accp.tile([48, 128], FP, tag="accs_sb")
                acc_v = accp.tile([48, 128], FP, tag="accv_sb")
                nc.scalar.activation(
                    acc_s[:], sc_s[:], mybir.ActivationFunctionType.Copy
                )
                nc.scalar.activation(
                    acc_v[:], sc_v[:], mybir.ActivationFunctionType.Copy
                )
                sT_ps = psp.tile([48, 128], FP, tag="ps")
                gT_ps = psp.tile([48, 128], FP, tag="ps")
                nsT_ps = psp.tile([48, 128], FP, tag="ps")
                nc.tensor.matmul(
                    sT_ps[:], lhsT=ws_sb[:], rhs=acc_s[:], start=True, stop=True
                )
                nc.tensor.matmul(
                    gT_ps[:], lhsT=wg_sb[:], rhs=acc_s[:], start=True, stop=True
                )
                nc.tensor.matmul(
                    nsT_ps[:], lhsT=wns_sb[:], rhs=acc_v[:], start=True, stop=True
                )
                sT = msgp.tile([48, 128], FP, tag="sT_sb")
                gT = msgp.tile([48, 128], FP, tag="gT_sb")
                fin = msgp.tile([48, 128], FP, tag="fin")
                nc.scalar.activation(
                    sT[:], sT_ps[:], mybir.ActivationFunctionType.Sigmoid
                )
                nc.vector.tensor_tensor(
                    out=sT[:], in0=sT_ps[:], in1=sT[:], op=mybir.AluOpType.mult
                )
                nc.scalar.activation(
                    gT[:], gT_ps[:], mybir.ActivationFunctionType.Sigmoid
                )
                nc.vector.tensor_tensor(
                    out=fin[:], in0=gT[:], in1=nsT_ps[:], op=mybir.AluOpType.mult
                )
                nc.vector.tensor_tensor(
                    out=fin[:], in0=fin[:], in1=sT[:], op=mybir.AluOpType.add
                )
                finT_ps = psp.tile([128, 48], FP, tag="ps")
                nc.tensor.transpose(finT_ps[:], fin[:], ident_sb[:48, :48])
                fino = msgp.tile([128, 48], FP, tag="fino")
                nc.scalar.activation(
                    fino[:], finT_ps[:], mybir.ActivationFunctionType.Copy
                )
                nc.sync.dma_start(
                    out=d_out[128 * b : 128 * (b + 1), :], in_=fino[:]
                )
          if rep_ctx is not None:
              rep_ctx.__exit__(None, None, None)
    nc.finalize()
    return nc


def _make_in_maps(inputs, srcidx, dstpos, dstloc):
    x = np.ascontiguousarray(np.asarray(inputs["x"], np.float32))
    pos = np.ascontiguousarray(np.asarray(inputs["pos"], np.float32))
    xcat = np.concatenate(
        [x, pos, np.zeros((N_NODES, 1), np.float32)], axis=1
    )  # [N, 100]
    posp = np.concatenate([pos, np.zeros((N_NODES, 1), np.float32)], axis=1)
    import ml_dtypes
    w2p = _permute_w2(np.asarray(inputs["w2"], np.float32)).astype(ml_dtypes.bfloat16)
    ws_c = (np.asarray(inputs["Ws"], np.float32) / np.sqrt(M0)).astype(np.float32)
    wg_c = (np.asarray(inputs["Wg"], np.float32) / np.sqrt(M0)).astype(np.float32)
    wns_c = _wns_block(np.asarray(inputs["Wns"], np.float32))
    w1 = np.ascontiguousarray(np.asarray(inputs["w1"], np.float32)).astype(ml_dtypes.bfloat16)
    ident = np.eye(128, dtype=np.float32)
    iota = np.tile(np.arange(128, dtype=np.float32), (128, 1)).astype(ml_dtypes.bfloat16)
    in_maps = []
    for c in range(N_CORES):
        in_maps.append({
            "xcat": xcat, "posp": posp,
            "srcidx": np.ascontiguousarray(srcidx[c]),
            "dstpos": np.ascontiguousarray(dstpos[c]),
            "dstloc": np.ascontiguousarray(dstloc[c]),
            "w1": w1, "w2p": w2p, "ws": ws_c, "wg": wg_c, "wns": wns_c,
            "ident": ident, "iota": iota,
        })
    return in_maps


def kernel(x, pos, edge_index, w1, w2, Ws, Wns, Wg):
    inputs = {"x": x, "pos": pos, "w1": w1, "w2": w2,
              "Ws": Ws, "Wns": Wns, "Wg": Wg}
    srcidx, dstpos, dstloc, tiles_per_bucket = _prep_edges(
        np.asarray(edge_index, np.int64)
    )
    in_maps = _make_in_maps(inputs, srcidx, dstpos, dstloc)
    nc = build_kernel(tiles_per_bucket)
    res = run_bass_kernel_spmd(nc, in_maps, core_ids=list(range(N_CORES)))
    return np.concatenate([res.results[c]["out"] for c in range(N_CORES)], axis=0)



# revision 20
# speedup vs baseline: 1.8947x; 1.8947x over previous
"""EquivariantEdgeConv fused Bass kernel for one TRN2 chip (8 NeuronCores).

Strategy (low-rank radial weights + fused TP/scatter on the PE):
  - The per-edge tensor-product weights w(len) = silu(len*w1) @ w2 / sqrt(H)
    lie on a 1-D curve parameterized by len.  An SVD of that curve (host
    side) shows rank R=6 reproduces it to ~1e-4, so each edge only needs
    R radial coefficients c[e,r] = h(len) @ V  (V = w2 @ B_R / sqrt(H)).
  - Per edge build a 256-wide feature row
        F = [ xs(48) | xv(48) | xv.y1(16) | xs x y1 (144) ]
    and the rank-1 expansion P[e, (r,f)] = c_r * F_f  (R*256 = 1536 cols,
    bf16, formed with R tensor_scalar ops at 4x DVE mode).
  - The tensor-product contraction is FUSED INTO THE SCATTER: per 128-edge
    tile, 12 PE matmuls accumulate Q[(r,f), n] += P_chunk^T @ onehot(dst)
    into PSUM; per 128-node bucket one [1536->96] weight contraction
    (12 accumulating matmuls against the host-built W_big) yields the
    scattered messages, followed by the gated o3.Linear node stage.
  - Nodes are sharded: core c owns nodes [1024c, 1024c+1024) as 8 buckets
    of 128; edges are bucketed by destination (host side) and padded to a
    shared static tile count.  x (+ a bf16-pair encoding of pos[src]) is
    fetched with ONE dma_gather per bucket; pos[dst] / dstloc / gather
    indices are streamed as contiguous per-bucket blocks.

Self-contained: shapes hardcoded for N=8192, E=65536, irreps 48x0e+16x1o,
H=64.
"""

import sys

if "/opt/trn_rl_repo" not in sys.path:
    sys.path.insert(0, "/opt/trn_rl_repo")

import numpy as np

import concourse.bacc as bacc
import concourse.bass as bass
import concourse.mybir as mybir
import concourse.tile as tile
from concourse.bass_utils import run_bass_kernel_spmd

M0, M1, H = 48, 16, 64
N_NODES, N_EDGES, N_CORES = 8192, 65536, 8
NODES_PER_CORE = N_NODES // N_CORES          # 1024
BUCKETS = NODES_PER_CORE // 128              # 8 buckets of 128 nodes per core
R = 6                                        # radial basis rank
GATHER_MODE = __import__('os').environ.get('GATHER_MODE', 'gather')  # gather|indirect
NF = 256                                     # per-edge feature width
NCHUNK = R * NF // 128                       # 12 P/W chunks of 128 rows
FP = mybir.dt.float32
BF = mybir.dt.bfloat16
I16 = mybir.dt.int16
I32 = mybir.dt.int32

CA = 1.0 / np.sqrt(M0 * 2.0)
CB = 1.0 / np.sqrt(3.0 * M1 * 2.0)
CC = 1.0 / np.sqrt(M0 * 2.0)
CD = 1.0 / np.sqrt(M1 * 2.0)
SQRT3 = float(np.sqrt(3.0))


def _silu64(x):
    return x / (1.0 + np.exp(-x))


def _radial_basis(w1, w2, len_max):
    """Rank-R SVD basis of the radial weight curve w(len).

    Returns V [H, R] (h -> coefficients) and BR [4096, R] (basis rows),
    both float64.
    """
    grid = np.linspace(0.0, float(len_max) * 1.02 + 1e-6, 1024)
    hs = _silu64(grid[:, None] * w1.astype(np.float64)[0][None, :])   # [S,H]
    ws = hs @ w2.astype(np.float64) / np.sqrt(H)                      # [S,4096]
    _, _, vt = np.linalg.svd(ws, full_matrices=False)
    br = vt[:R].T                                                     # [4096,R]
    v = w2.astype(np.float64) @ br / np.sqrt(H)                       # [H,R]
    return v, br


def _build_wbig(br):
    """W_big [(r,f), q] mapping rank-1 features to the 96 message outputs.

    f layout: [xs(48) | xv(i,m)(48) | xvy(16) | xsY(m-major,144)]
    q layout: [ms o<48 | pad(16) | mv 64+3o+m | pad(16)]  (mv at partition
    base 64 so the epilogue ACT copy reads at a legal partition offset)
    Returns [R*256, 128] float64.
    """
    wb = np.zeros((R, NF, 128))
    # a/b/c/d carry the trailing R axis ([i, o, R])
    a = br[:2304].reshape(48, 48, R)
    b = br[2304:3072].reshape(16, 48, R)
    c = br[3072:3840].reshape(48, 16, R)
    d = br[3840:4096].reshape(16, 16, R)
    for r in range(R):
        # path A: f=i (xs), q=o
        wb[r, 0:48, 0:48] = CA * a[:, :, r]
        # path B: f=96+i (xvy), q=o  (sqrt3 from Y1)
        wb[r, 96:112, 0:48] = CB * SQRT3 * b[:, :, r]
        # path D: f=48+3i+m (xv), q=64+3o+m
        for m in range(3):
            wb[r, 48 + m:96:3, 64 + m:112:3] = CD * d[:, :, r]
            # path C: f=112+48m+i (xsY), q=64+3o+m  (sqrt3 from Y1)
            wb[r, 112 + 48 * m:160 + 48 * m, 64 + m:112:3] = CC * SQRT3 * c[:, :, r]
    return wb.reshape(R * NF, 128)


def _wns_block(wns):
    """[48,48] lhsT for the 1o o3.Linear on (o,m)-interleaved rows."""
    out = np.zeros((48, 48), np.float32)
    for i in range(16):
        for m in range(3):
            for o in range(16):
                out[i * 3 + m, o * 3 + m] = wns[i, o] / np.sqrt(M1)
    return out


def _prep_edges(edge_index, pos):
    """Bucket/pad edges by destination.

    Returns per-core arrays:
      idx16  [N_CORES, BUCKETS*128, T*8]  int16  (dma_gather wrapped+replicated)
      dl     [N_CORES, BUCKETS*128, T]    fp32   (local dst, 300 for padding)
      pdst   [N_CORES, BUCKETS*128, T*4]  fp32   (pos[dst], w-padded)
    and the shared tiles-per-bucket count T.
    """
    src = edge_index[0].astype(np.int64)
    dst = edge_index[1].astype(np.int64)
    gb = dst >> 7
    order = np.argsort(gb, kind="stable")
    src_s, dst_s = src[order], dst[order]
    counts = np.bincount(gb[order], minlength=64)
    cap = max(int(np.ceil(counts.max() / 128) * 128), 128)
    T = cap // 128
    starts = np.concatenate([[0], np.cumsum(counts)])

    pos = np.asarray(pos, np.float32)
    idx16 = np.zeros((N_CORES, BUCKETS * 128, T * 8), np.int16)
    srcidx = np.zeros((N_CORES, BUCKETS * 128, T), np.int32)
    dl = np.full((N_CORES, BUCKETS * 128, T), 300.0, np.float32)
    pdst = np.zeros((N_CORES, BUCKETS * 128, T * 4), np.float32)

    for g in range(64):
        ccore, b = g >> 3, g & 7
        s, e = starts[g], starts[g + 1]
        n = e - s
        sidx = np.zeros(cap, np.int64)
        sidx[:n] = src_s[s:e]
        dloc = np.full(cap, 300.0, np.float32)
        dloc[:n] = (dst_s[s:e] - (g << 7)).astype(np.float32)
        pd = np.zeros((cap, 3), np.float32)
        pd[:n] = pos[dst_s[s:e]]
        pd[n:] = pos[0]  # padding: same as pos[src=0] so vec==0, no NaNs
        # edge k -> partition k%128, tile k//128
        k = np.arange(cap)
        p, t = k % 128, k // 128
        r0 = 128 * b
        dl[ccore, r0 + p, t] = dloc
        srcidx[ccore, r0 + p, t] = sidx.astype(np.int32)
        pdst[ccore, r0 + p[:, None], 4 * t[:, None] + np.arange(3)[None, :]] = pd
        # gather idx wrap: idx k -> [k%16, k//16], replicated to 128 partitions
        wrapped = np.zeros((16, T * 8), np.int16)
        wrapped[k % 16, k // 16] = sidx.astype(np.int16)
        idx16[ccore, r0:r0 + 128, :] = np.tile(wrapped, (8, 1))
    return (idx16, srcidx), dl, pdst, T


def build_kernel(tiles_per_bucket: int, reps: int = 1) -> bass.Bass:
    T = tiles_per_bucket
    nc = bacc.Bacc(None, target_bir_lowering=False, debug=False)
    d_xb = nc.declare_dram_parameter("xb", [N_NODES, 128], BF, isOutput=False)
    d_idx = nc.declare_dram_parameter("idx16", [BUCKETS * 128, T * 8], I16, isOutput=False)
    d_srcidx = nc.declare_dram_parameter("srcidx", [BUCKETS * 128, T], I32, isOutput=False)
    d_dl = nc.declare_dram_parameter("dl", [BUCKETS * 128, T], FP, isOutput=False)
    d_pd = nc.declare_dram_parameter("pdst", [BUCKETS * 128, T * 4], FP, isOutput=False)
    d_wbig = nc.declare_dram_parameter("wbig", [128, NCHUNK * 128], BF, isOutput=False)
    d_v = nc.declare_dram_parameter("vmat", [H, R], FP, isOutput=False)
    d_w1 = nc.declare_dram_parameter("w1", [1, H], FP, isOutput=False)
    d_ws = nc.declare_dram_parameter("ws", [M0, M0], FP, isOutput=False)
    d_wg = nc.declare_dram_parameter("wg", [M0, M0], FP, isOutput=False)
    d_wns = nc.declare_dram_parameter("wns", [48, 48], FP, isOutput=False)
    d_ident = nc.declare_dram_parameter("ident", [128, 128], FP, isOutput=False)
    d_iota = nc.declare_dram_parameter("iota", [128, 128], BF, isOutput=False)
    d_zero = nc.declare_dram_parameter("zrow", [1, 512], FP, isOutput=False)
    d_out = nc.declare_dram_parameter("out", [NODES_PER_CORE, M0], FP, isOutput=True)

    AF = mybir.ActivationFunctionType
    OP = mybir.AluOpType

    with tile.TileContext(nc) as tc, tc.tile_pool(name="consts", bufs=1) as cp:
        wbig_sb = cp.tile([128, NCHUNK * 128], BF)
        v_sb = cp.tile([H, R], FP)
        w1_sb = cp.tile([1, H], FP)
        ws_sb = cp.tile([M0, M0], FP)
        wg_sb = cp.tile([M0, M0], FP)
        wns_sb = cp.tile([48, 48], FP)
        ident_sb = cp.tile([128, 128], FP)
        iota_sb = cp.tile([128, 128], BF)
        zrow_sb = cp.tile([1, 512], FP)
        for sb, dr in (
            (wbig_sb, d_wbig), (v_sb, d_v), (w1_sb, d_w1), (ws_sb, d_ws),
            (wg_sb, d_wg), (wns_sb, d_wns), (ident_sb, d_ident), (iota_sb, d_iota),
            (zrow_sb, d_zero),
        ):
            nc.sync.dma_start(out=sb[:], in_=dr[:])

        with (
            tc.tile_pool(name="bkt", bufs=2) as bktp,
            tc.tile_pool(name="geo", bufs=3) as geop,
            tc.tile_pool(name="feat", bufs=3) as featp,
            tc.tile_pool(name="pbuf", bufs=2) as pbufp,
            tc.tile_pool(name="epi", bufs=2) as epip,
            tc.tile_pool(name="qacc", bufs=1, space="PSUM") as qaccp,
            tc.tile_pool(name="ps_small", bufs=2, space="PSUM") as psp,
            tc.tile_pool(name="ps_epi", bufs=1, space="PSUM") as pse,
        ):
            rep_ctx = tc.For_i(0, reps, 1) if reps > 1 else None
            if rep_ctx is not None:
                rep_ctx.__enter__()
            for b in range(BUCKETS):
                dlb = bktp.tile([128, T], FP, tag="dl")
                pdb = bktp.tile([128, T * 4], FP, tag="pd")
                xgb = bktp.tile([128, T * 128], BF, tag="xgb")
                r0 = 128 * b
                nc.sync.dma_start(out=dlb[:], in_=d_dl[r0:r0 + 128, :])
                nc.sync.dma_start(out=pdb[:], in_=d_pd[r0:r0 + 128, :])
                if GATHER_MODE == "gather":
                    idxt = bktp.tile([128, T * 8], I16, tag="idx")
                    nc.sync.dma_start(out=idxt[:], in_=d_idx[r0:r0 + 128, :])
                    nc.gpsimd.dma_gather(
                        out_ap=xgb[:].rearrange("p (t e) -> p t e", e=128),
                        in_ap=d_xb[:, :],
                        idxs_ap=idxt[:],
                        num_idxs=T * 128,
                        num_idxs_reg=T * 128,
                        elem_size=128,
                    )
                else:
                    sidxt = bktp.tile([128, T], I32, tag="sidx")
                    nc.sync.dma_start(out=sidxt[:], in_=d_srcidx[r0:r0 + 128, :])
                    for tt in range(T):
                        nc.gpsimd.indirect_dma_start(
                            out=xgb[:, 128 * tt:128 * (tt + 1)],
                            out_offset=None,
                            in_=d_xb[:],
                            in_offset=bass.IndirectOffsetOnAxis(
                                ap=sidxt[:, tt:tt + 1], axis=0
                            ),
                        )
                q_ps = [
                    qaccp.tile([128, 512], FP, tag=f"q{j}", name=f"q_ps{j}")
                    for j in range(3)
                ]
                # Claim each accumulator bank with one zeroing matmul
                # (start=True clears has_written for the whole bank); the 12
                # per-tile chunk matmuls then accumulate with start=False.
                for j in range(3):
                    nc.tensor.matmul(
                        q_ps[j][:],
                        lhsT=ident_sb[0:1, :],
                        rhs=zrow_sb[0:1, :],
                        start=True,
                        stop=False,
                    )
                for t in range(T):
                    xt = xgb[:, 128 * t:128 * (t + 1)]
                    pd3 = pdb[:, 4 * t:4 * t + 3]
                    dlc = dlb[:, t:t + 1]
                    # ---- geometry ----
                    vec = geop.tile([128, 3], FP, tag="vec")
                    sqj = geop.tile([128, 3], FP, tag="sqj")
                    lensq = geop.tile([128, 1], FP, tag="lensq")
                    lenc = geop.tile([128, 1], FP, tag="lenc")
                    invl = geop.tile([128, 1], FP, tag="invl")
                    y1 = geop.tile([128, 3], FP, tag="y1")
                    nc.vector.tensor_tensor(
                        out=vec[:], in0=pd3, in1=xt[:, 96:99], op=OP.subtract
                    )
                    nc.vector.tensor_tensor(
                        out=vec[:], in0=vec[:], in1=xt[:, 99:102], op=OP.subtract
                    )
                    nc.scalar.activation(
                        sqj[:], vec[:], AF.Square, accum_out=lensq[:]
                    )
                    nc.scalar.activation(lenc[:], lensq[:], AF.Sqrt)
                    nc.vector.tensor_scalar_max(lenc[:], lenc[:], 1e-8)
                    nc.vector.reciprocal(invl[:], lenc[:])
                    nc.vector.tensor_scalar_mul(y1[:], vec[:], invl[:, :1])
                    # ---- radial coefficients c[e, R] ----
                    # one PSUM bank per tile iteration, sub-viewed
                    mix_ps = psp.tile([128, 512], FP, tag="mix")
                    lenT_ps = mix_ps[0:1, 0:128]
                    hpre_ps = mix_ps[0:64, 128:256]
                    c_ps = mix_ps[:, 256:256 + R]
                    nc.tensor.matmul(
                        lenT_ps, lhsT=lenc[:], rhs=ident_sb[:],
                        start=True, stop=True,
                    )
                    lenrow = geop.tile([1, 128], FP, tag="lenrow")
                    nc.scalar.activation(lenrow[:], lenT_ps, AF.Copy)
                    nc.tensor.matmul(
                        hpre_ps, lhsT=w1_sb[:1, :], rhs=lenrow[:1, :],
                        start=True, stop=True,
                    )
                    hsig = geop.tile([64, 128], FP, tag="hsig")
                    hT = geop.tile([64, 128], FP, tag="hT")
                    nc.scalar.activation(hsig[:], hpre_ps, AF.Sigmoid)
                    nc.vector.tensor_tensor(
                        out=hT[:], in0=hpre_ps, in1=hsig[:], op=OP.mult
                    )
                    nc.tensor.matmul(
                        c_ps, lhsT=hT[:], rhs=v_sb[:], start=True, stop=True
                    )
                    c_sb = geop.tile([128, R], FP, tag="csb")
                    nc.scalar.activation(c_sb[:], c_ps, AF.Copy)
                    # ---- features F = [xs | xv | xvy | xsY] ----
                    ft = featp.tile([128, NF], BF, tag="F")
                    pvy = featp.tile([128, 48], FP, tag="pvy")
                    nc.vector.tensor_copy(ft[:, 0:96], xt[:, 0:96])
                    nc.vector.tensor_tensor(
                        out=pvy[:].rearrange("p (i m) -> p i m", m=3),
                        in0=xt[:, 48:96].rearrange("p (i m) -> p i m", m=3),
                        in1=y1[:].rearrange("p (o m) -> p o m", o=1).to_broadcast(
                            [128, 16, 3]
                        ),
                        op=OP.mult,
                    )
                    with nc.allow_low_precision(reason="3-term dot, bf16 out"):
                        nc.vector.reduce_sum(
                            ft[:, 96:112],
                            pvy[:].rearrange("p (i m) -> p i m", m=3),
                            axis=mybir.AxisListType.X,
                        )
                    for m in range(3):
                        nc.vector.tensor_scalar_mul(
                            ft[:, 112 + 48 * m:160 + 48 * m],
                            xt[:, 0:48],
                            y1[:, m:m + 1],
                        )
                    # ---- rank-1 expansion P[e, (r,f)] ----
                    pt = pbufp.tile([128, R * NF], BF, tag="P")
                    for r in range(R):
                        nc.vector.tensor_scalar_mul(
                            pt[:, NF * r:NF * (r + 1)], ft[:], c_sb[:, r:r + 1]
                        )
                    # ---- one-hot + fused TP/scatter ----
                    oh = featp.tile([128, 128], BF, tag="oh")
                    nc.vector.tensor_scalar(
                        out=oh[:], in0=iota_sb[:], scalar1=dlc[:, :1],
                        scalar2=None, op0=OP.is_equal,
                    )
                    for k in range(NCHUNK):
                        nc.tensor.matmul(
                            q_ps[k // 4][:, 128 * (k % 4):128 * (k % 4 + 1)],
                            lhsT=pt[:, 128 * k:128 * (k + 1)],
                            rhs=oh[:],
                            start=False,
                            stop=(t == T - 1 and (k % 4) == 3),
                        )
                # ---- bucket epilogue: weight contraction + node stage ----
                qsb = epip.tile([128, NCHUNK * 128], BF, tag="qsb")
                for j in range(3):
                    nc.scalar.activation(
                        qsb[:, 512 * j:512 * (j + 1)], q_ps[j][:], AF.Copy
                    )
                big_ps = pse.tile([128, 512], FP, tag="big")
                outT_ps = big_ps[:, 0:128]
                sT_ps = big_ps[0:48, 128:256]
                gT_ps = big_ps[0:48, 256:384]
                nsT_ps = big_ps[0:48, 384:512]
                for k in range(NCHUNK):
                    nc.tensor.matmul(
                        outT_ps,
                        lhsT=wbig_sb[:, 128 * k:128 * (k + 1)],
                        rhs=qsb[:, 128 * k:128 * (k + 1)],
                        start=(k == 0),
                        stop=(k == NCHUNK - 1),
                    )
                acc_s = epip.tile([48, 128], FP, tag="acc_s")
                acc_v = epip.tile([48, 128], FP, tag="acc_v")
                nc.scalar.activation(acc_s[:], outT_ps[0:48, :], AF.Copy)
                nc.scalar.activation(acc_v[:], outT_ps[64:112, :], AF.Copy)
                nc.tensor.matmul(
                    sT_ps, lhsT=ws_sb[:], rhs=acc_s[:], start=True, stop=True
                )
                nc.tensor.matmul(
                    gT_ps, lhsT=wg_sb[:], rhs=acc_s[:], start=True, stop=True
                )
                nc.tensor.matmul(
                    nsT_ps, lhsT=wns_sb[:], rhs=acc_v[:], start=True, stop=True
                )
                sT = epip.tile([48, 128], FP, tag="sT_sb")
                gT = epip.tile([48, 128], FP, tag="gT_sb")
                fin = epip.tile([48, 128], FP, tag="fin")
                nc.scalar.activation(sT[:], sT_ps, AF.Sigmoid)
                nc.scalar.activation(gT[:], gT_ps, AF.Sigmoid)
                nc.vector.tensor_tensor(
                    out=sT[:], in0=sT_ps, in1=sT[:], op=OP.mult
                )
                nc.vector.tensor_tensor(
                    out=fin[:], in0=gT[:], in1=nsT_ps, op=OP.mult
                )
                nc.vector.tensor_tensor(
                    out=fin[:], in0=fin[:], in1=sT[:], op=OP.add
                )
                finT_ps = pse.tile([128, 48], FP, tag="finT")
                nc.tensor.transpose(finT_ps[:], fin[:], ident_sb[:48, :48])
                fino = epip.tile([128, 48], FP, tag="fino")
                nc.scalar.activation(fino[:], finT_ps[:], AF.Copy)
                nc.sync.dma_start(
                    out=d_out[128 * b:128 * (b + 1), :], in_=fino[:]
                )
            if rep_ctx is not None:
                rep_ctx.__exit__(None, None, None)
    nc.finalize()
    return nc


def _make_in_maps(inputs, idx_pair, dl, pdst):
    idx16, srcidx = idx_pair
    import ml_dtypes

    x = np.asarray(inputs["x"], np.float32)
    pos = np.asarray(inputs["pos"], np.float32)
    w1 = np.asarray(inputs["w1"], np.float32)
    w2 = np.asarray(inputs["w2"], np.float32)

    # node table: [x bf16 (96) | pos hi (3) | pos lo (3) | 0 pad] = 128 bf16
    xb = np.zeros((N_NODES, 128), ml_dtypes.bfloat16)
    xb[:, 0:96] = x.astype(ml_dtypes.bfloat16)
    poshi = pos.astype(ml_dtypes.bfloat16)
    poslo = (pos - poshi.astype(np.float32)).astype(ml_dtypes.bfloat16)
    xb[:, 96:99] = poshi
    xb[:, 99:102] = poslo

    # len_max from actual edge geometry (host gather, cheap)
    ei = np.asarray(inputs["edge_index"], np.int64)
    vec = pos[ei[1]] - pos[ei[0]]
    len_max = float(np.sqrt((vec * vec).sum(axis=1)).max())

    v, br = _radial_basis(w1, w2, len_max)
    wbig = _build_wbig(br)                          # [R*256, 128]
    wbig_packed = np.ascontiguousarray(
        wbig.reshape(NCHUNK, 128, 128).transpose(1, 0, 2).reshape(128, NCHUNK * 128)
    ).astype(ml_dtypes.bfloat16)

    ws_c = (np.asarray(inputs["Ws"], np.float32) / np.sqrt(M0)).astype(np.float32)
    wg_c = (np.asarray(inputs["Wg"], np.float32) / np.sqrt(M0)).astype(np.float32)
    wns_c = _wns_block(np.asarray(inputs["Wns"], np.float32))
    ident = np.eye(128, dtype=np.float32)
    iota = np.tile(np.arange(128, dtype=np.float32), (128, 1)).astype(
        ml_dtypes.bfloat16
    )
    in_maps = []
    for c in range(N_CORES):
        in_maps.append({
            "xb": xb,
            "idx16": np.ascontiguousarray(idx16[c]),
            "srcidx": np.ascontiguousarray(srcidx[c]),
            "dl": np.ascontiguousarray(dl[c]),
            "pdst": np.ascontiguousarray(pdst[c]),
            "wbig": wbig_packed,
            "vmat": v.astype(np.float32),
            "w1": w1.astype(np.float32),
            "ws": ws_c, "wg": wg_c, "wns": wns_c,
            "ident": ident, "iota": iota,
            "zrow": np.zeros((1, 512), np.float32),
        })
    return in_maps


def kernel(x, pos, edge_index, w1, w2, Ws, Wns, Wg):
    inputs = {"x": x, "pos": pos, "edge_index": edge_index, "w1": w1,
              "w2": w2, "Ws": Ws, "Wns": Wns, "Wg": Wg}
    idx16, dl, pdst, T = _prep_edges(
        np.asarray(edge_index, np.int64), np.asarray(pos, np.float32)
    )
    in_maps = _make_in_maps(inputs, idx16, dl, pdst)
    nc = build_kernel(T)
    res = run_bass_kernel_spmd(nc, in_maps, core_ids=list(range(N_CORES)))
    return np.concatenate([res.results[c]["out"] for c in range(N_CORES)], axis=0)


# revision 23
# speedup vs baseline: 2.2690x; 1.1975x over previous
"""EquivariantEdgeConv fused Bass kernel for one TRN2 chip (8 NeuronCores).

Strategy (low-rank radial weights + fused TP/scatter on the PE):
  - The per-edge tensor-product weights w(len) = silu(len*w1) @ w2 / sqrt(H)
    lie on a 1-D curve parameterized by len.  An SVD of that curve (host
    side) shows rank R=6 reproduces it to ~1e-4, so each edge only needs
    R radial coefficients c[e,r] = h(len) @ V  (V = w2 @ B_R / sqrt(H)).
  - Per edge build a 256-wide feature row
        F = [ xs(48) | xv(48) | xv.y1(16) | xs x y1 (144) ]
    and the rank-1 expansion P[e, (r,f)] = c_r * F_f  (R*256 = 1536 cols,
    bf16, formed with R tensor_scalar ops at 4x DVE mode).
  - The tensor-product contraction is FUSED INTO THE SCATTER: per 128-edge
    tile, 12 PE matmuls accumulate Q[(r,f), n] += P_chunk^T @ onehot(dst)
    into PSUM; per 128-node bucket one [1536->96] weight contraction
    (12 accumulating matmuls against the host-built W_big) yields the
    scattered messages, followed by the gated o3.Linear node stage.
  - Nodes are sharded: core c owns nodes [1024c, 1024c+1024) as 8 buckets
    of 128; edges are bucketed by destination (host side) and padded to a
    shared static tile count.  x (+ a bf16-pair encoding of pos[src]) is
    fetched with ONE dma_gather per bucket; pos[dst] / dstloc / gather
    indices are streamed as contiguous per-bucket blocks.

Self-contained: shapes hardcoded for N=8192, E=65536, irreps 48x0e+16x1o,
H=64.
"""

import sys

if "/opt/trn_rl_repo" not in sys.path:
    sys.path.insert(0, "/opt/trn_rl_repo")

import numpy as np

import concourse.bacc as bacc
import concourse.bass as bass
import concourse.mybir as mybir
import concourse.tile as tile
from concourse.bass_utils import run_bass_kernel_spmd

M0, M1, H = 48, 16, 64
N_NODES, N_EDGES, N_CORES = 8192, 65536, 8
NODES_PER_CORE = N_NODES // N_CORES          # 1024
BUCKETS = NODES_PER_CORE // 128              # 8 buckets of 128 nodes per core
R = 6                                        # radial basis rank
GATHER_MODE = __import__('os').environ.get('GATHER_MODE', 'gather')  # gather|indirect
NF = 256                                     # per-edge feature width
NCHUNK = R * NF // 128                       # 12 P/W chunks of 128 rows
FP = mybir.dt.float32
BF = mybir.dt.bfloat16
I16 = mybir.dt.int16
I32 = mybir.dt.int32

CA = 1.0 / np.sqrt(M0 * 2.0)
CB = 1.0 / np.sqrt(3.0 * M1 * 2.0)
CC = 1.0 / np.sqrt(M0 * 2.0)
CD = 1.0 / np.sqrt(M1 * 2.0)
SQRT3 = float(np.sqrt(3.0))


def _silu64(x):
    return x / (1.0 + np.exp(-x))


NKNOT = 64


def _radial_basis(w1, w2, len_max):
    """Rank-R SVD basis of the radial weight curve w(len), evaluated on
    device through a ReLU spline: c_r(len) = relu(len - knots) @ A.

    (ReLU is used because sqrt/relu/square/copy share one ACT function
    table set on TRN2 -- no per-tile table reloads.)

    Returns knots [NKNOT], A [NKNOT, R], BR [4096, R], all float64.
    """
    grid = np.linspace(0.0, float(len_max) * 1.02 + 1e-6, 2048)
    hs = _silu64(grid[:, None] * w1.astype(np.float64)[0][None, :])   # [S,H]
    ws = hs @ w2.astype(np.float64) / np.sqrt(H)                      # [S,4096]
    _, _, vt = np.linalg.svd(ws, full_matrices=False)
    br = vt[:R].T                                                     # [4096,R]
    c_true = ws @ br                                                  # [S,R]
    knots = np.linspace(0.0, float(len_max) * 1.01, NKNOT)
    g = np.maximum(grid[:, None] - knots[None, :], 0.0)               # [S,NKNOT]
    a = np.linalg.solve(
        g.T @ g + 1e-7 * np.eye(NKNOT), g.T @ c_true
    )                                                                 # [NKNOT,R]
    return knots, a, br


def _build_wbig(br):
    """W_big [(r,f), q] mapping rank-1 features to the 96 message outputs.

    f layout: [xs(48) | xv(i,m)(48) | xvy(16) | xsY(m-major,144)]
    q layout: [ms o<48 | pad(16) | mv 64+3o+m | pad(16)]  (mv at partition
    base 64 so the epilogue ACT copy reads at a legal partition offset)
    Returns [R*256, 128] float64.
    """
    wb = np.zeros((R, NF, 128))
    # a/b/c/d carry the trailing R axis ([i, o, R])
    a = br[:2304].reshape(48, 48, R)
    b = br[2304:3072].reshape(16, 48, R)
    c = br[3072:3840].reshape(48, 16, R)
    d = br[3840:4096].reshape(16, 16, R)
    for r in range(R):
        # path A: f=i (xs), q=o
        wb[r, 0:48, 0:48] = CA * a[:, :, r]
        # path B: f=96+i (xvy), q=o  (sqrt3 from Y1)
        wb[r, 96:112, 0:48] = CB * SQRT3 * b[:, :, r]
        # path D: f=48+3i+m (xv), q=64+3o+m
        for m in range(3):
            wb[r, 48 + m:96:3, 64 + m:112:3] = CD * d[:, :, r]
            # path C: f=112+48m+i (xsY), q=64+3o+m  (sqrt3 from Y1)
            wb[r, 112 + 48 * m:160 + 48 * m, 64 + m:112:3] = CC * SQRT3 * c[:, :, r]
    return wb.reshape(R * NF, 128)


def _wns_block(wns):
    """[48,48] lhsT for the 1o o3.Linear on (o,m)-interleaved rows."""
    out = np.zeros((48, 48), np.float32)
    for i in range(16):
        for m in range(3):
            for o in range(16):
                out[i * 3 + m, o * 3 + m] = wns[i, o] / np.sqrt(M1)
    return out


def _prep_edges(edge_index, pos):
    """Bucket/pad edges by destination.

    Returns per-core arrays:
      idx16  [N_CORES, BUCKETS*128, T*8]  int16  (dma_gather wrapped+replicated)
      dl     [N_CORES, BUCKETS*128, T]    fp32   (local dst, 300 for padding)
      pdst   [N_CORES, BUCKETS*128, T*4]  fp32   (pos[dst], w-padded)
    and the shared tiles-per-bucket count T.
    """
    src = edge_index[0].astype(np.int64)
    dst = edge_index[1].astype(np.int64)
    gb = dst >> 7
    order = np.argsort(gb, kind="stable")
    src_s, dst_s = src[order], dst[order]
    counts = np.bincount(gb[order], minlength=64)
    cap = max(int(np.ceil(counts.max() / 128) * 128), 128)
    T = cap // 128
    starts = np.concatenate([[0], np.cumsum(counts)])

    pos = np.asarray(pos, np.float32)
    idx16 = np.zeros((N_CORES, BUCKETS * 128, T * 8), np.int16)
    srcidx = np.zeros((N_CORES, BUCKETS * 128, T), np.int32)
    dl = np.full((N_CORES, BUCKETS * 128, T), 300.0, np.float32)
    pdst = np.zeros((N_CORES, BUCKETS * 128, T * 4), np.float32)

    for g in range(64):
        ccore, b = g >> 3, g & 7
        s, e = starts[g], starts[g + 1]
        n = e - s
        sidx = np.zeros(cap, np.int64)
        sidx[:n] = src_s[s:e]
        dloc = np.full(cap, 300.0, np.float32)
        dloc[:n] = (dst_s[s:e] - (g << 7)).astype(np.float32)
        pd = np.zeros((cap, 3), np.float32)
        pd[:n] = pos[dst_s[s:e]]
        pd[n:] = pos[0]  # padding: same as pos[src=0] so vec==0, no NaNs
        # edge k -> partition k%128, tile k//128
        k = np.arange(cap)
        p, t = k % 128, k // 128
        r0 = 128 * b
        dl[ccore, r0 + p, t] = dloc
        srcidx[ccore, r0 + p, t] = sidx.astype(np.int32)
        pdst[ccore, r0 + p[:, None], 4 * t[:, None] + np.arange(3)[None, :]] = pd
        # gather idx wrap: idx k -> [k%16, k//16], replicated to 128 partitions
        wrapped = np.zeros((16, T * 8), np.int16)
        wrapped[k % 16, k // 16] = sidx.astype(np.int16)
        idx16[ccore, r0:r0 + 128, :] = np.tile(wrapped, (8, 1))
    return (idx16, srcidx), dl, pdst, T


def build_kernel(tiles_per_bucket: int, reps: int = 1) -> bass.Bass:
    T = tiles_per_bucket
    nc = bacc.Bacc(None, target_bir_lowering=False, debug=False)
    d_xb = nc.declare_dram_parameter("xb", [N_NODES, 128], BF, isOutput=False)
    d_idx = nc.declare_dram_parameter("idx16", [BUCKETS * 128, T * 8], I16, isOutput=False)
    d_srcidx = nc.declare_dram_parameter("srcidx", [BUCKETS * 128, T], I32, isOutput=False)
    d_dl = nc.declare_dram_parameter("dl", [BUCKETS * 128, T], FP, isOutput=False)
    d_pd = nc.declare_dram_parameter("pdst", [BUCKETS * 128, T * 4], FP, isOutput=False)
    d_wbig = nc.declare_dram_parameter("wbig", [128, NCHUNK * 128], BF, isOutput=False)
    d_v = nc.declare_dram_parameter("vmat", [NKNOT, R], FP, isOutput=False)
    d_ones = nc.declare_dram_parameter("onesrow", [1, NKNOT], FP, isOutput=False)
    d_knots = nc.declare_dram_parameter("nknots", [NKNOT, 1], FP, isOutput=False)
    d_ws = nc.declare_dram_parameter("ws", [M0, M0], FP, isOutput=False)
    d_wg = nc.declare_dram_parameter("wg", [M0, M0], FP, isOutput=False)
    d_wns = nc.declare_dram_parameter("wns", [48, 48], FP, isOutput=False)
    d_ident = nc.declare_dram_parameter("ident", [128, 128], FP, isOutput=False)
    d_iota = nc.declare_dram_parameter("iota", [128, 128], BF, isOutput=False)
    d_zero = nc.declare_dram_parameter("zrow", [1, 512], FP, isOutput=False)
    d_out = nc.declare_dram_parameter("out", [NODES_PER_CORE, M0], FP, isOutput=True)

    AF = mybir.ActivationFunctionType
    OP = mybir.AluOpType

    with tile.TileContext(nc) as tc, tc.tile_pool(name="consts", bufs=1) as cp:
        wbig_sb = cp.tile([128, NCHUNK * 128], BF)
        v_sb = cp.tile([NKNOT, R], FP)
        ones_sb = cp.tile([1, NKNOT], FP)
        knots_sb = cp.tile([NKNOT, 1], FP)
        ws_sb = cp.tile([M0, M0], FP)
        wg_sb = cp.tile([M0, M0], FP)
        wns_sb = cp.tile([48, 48], FP)
        ident_sb = cp.tile([128, 128], FP)
        iota_sb = cp.tile([128, 128], BF)
        zrow_sb = cp.tile([1, 512], FP)
        for sb, dr in (
            (wbig_sb, d_wbig), (v_sb, d_v), (ones_sb, d_ones),
            (knots_sb, d_knots), (ws_sb, d_ws),
            (wg_sb, d_wg), (wns_sb, d_wns), (ident_sb, d_ident), (iota_sb, d_iota),
            (zrow_sb, d_zero),
        ):
            nc.sync.dma_start(out=sb[:], in_=dr[:])

        with (
            tc.tile_pool(name="bkt", bufs=2) as bktp,
            tc.tile_pool(name="geo", bufs=3) as geop,
            tc.tile_pool(name="feat", bufs=3) as featp,
            tc.tile_pool(name="pbuf", bufs=2) as pbufp,
            tc.tile_pool(name="epi", bufs=2) as epip,
            tc.tile_pool(name="node", bufs=1) as nodep,
            tc.tile_pool(name="qacc", bufs=1, space="PSUM") as qaccp,
            tc.tile_pool(name="ps_small", bufs=2, space="PSUM") as psp,
            tc.tile_pool(name="ps_epi", bufs=1, space="PSUM") as pse,
        ):
            rep_ctx = tc.For_i(0, reps, 1) if reps > 1 else None
            if rep_ctx is not None:
                rep_ctx.__enter__()
            sT_all = nodep.tile([48, 1024], FP, tag="sT_all")
            gT_all = nodep.tile([48, 1024], FP, tag="gT_all")
            ns_all = nodep.tile([48, 1024], FP, tag="ns_all")
            for b in range(BUCKETS):
                dlb = bktp.tile([128, T], FP, tag="dl")
                pdb = bktp.tile([128, T * 4], FP, tag="pd")
                xgb = bktp.tile([128, T * 128], BF, tag="xgb")
                r0 = 128 * b
                nc.sync.dma_start(out=dlb[:], in_=d_dl[r0:r0 + 128, :])
                nc.sync.dma_start(out=pdb[:], in_=d_pd[r0:r0 + 128, :])
                if GATHER_MODE == "gather":
                    idxt = bktp.tile([128, T * 8], I16, tag="idx")
                    nc.sync.dma_start(out=idxt[:], in_=d_idx[r0:r0 + 128, :])
                    nc.gpsimd.dma_gather(
                        out_ap=xgb[:].rearrange("p (t e) -> p t e", e=128),
                        in_ap=d_xb[:, :],
                        idxs_ap=idxt[:],
                        num_idxs=T * 128,
                        num_idxs_reg=T * 128,
                        elem_size=128,
                        single_packet=False,
                    )
                else:
                    sidxt = bktp.tile([128, T], I32, tag="sidx")
                    nc.sync.dma_start(out=sidxt[:], in_=d_srcidx[r0:r0 + 128, :])
                    for tt in range(T):
                        nc.gpsimd.indirect_dma_start(
                            out=xgb[:, 128 * tt:128 * (tt + 1)],
                            out_offset=None,
                            in_=d_xb[:],
                            in_offset=bass.IndirectOffsetOnAxis(
                                ap=sidxt[:, tt:tt + 1], axis=0
                            ),
                        )
                q_ps = [
                    qaccp.tile([128, 512], FP, tag=f"q{j}", name=f"q_ps{j}")
                    for j in range(3)
                ]
                # Claim each accumulator bank with one zeroing matmul
                # (start=True clears has_written for the whole bank); the 12
                # per-tile chunk matmuls then accumulate with start=False.
                for j in range(3):
                    nc.tensor.matmul(
                        q_ps[j][:],
                        lhsT=ident_sb[0:1, :],
                        rhs=zrow_sb[0:1, :],
                        start=True,
                        stop=False,
                    )
                for t in range(T):
                    xt = xgb[:, 128 * t:128 * (t + 1)]
                    pd3 = pdb[:, 4 * t:4 * t + 3]
                    dlc = dlb[:, t:t + 1]
                    # ---- geometry ----
                    vec = geop.tile([128, 3], FP, tag="vec")
                    sqj = geop.tile([128, 3], FP, tag="sqj")
                    lensq = geop.tile([128, 1], FP, tag="lensq")
                    lenc = geop.tile([128, 1], FP, tag="lenc")
                    invl = geop.tile([128, 1], FP, tag="invl")
                    y1 = geop.tile([128, 3], FP, tag="y1")
                    nc.vector.tensor_tensor(
                        out=vec[:], in0=pd3, in1=xt[:, 96:99], op=OP.subtract
                    )
                    nc.vector.tensor_tensor(
                        out=vec[:], in0=vec[:], in1=xt[:, 99:102], op=OP.subtract
                    )
                    nc.scalar.activation(
                        sqj[:], vec[:], AF.Square, accum_out=lensq[:]
                    )
                    nc.scalar.activation(lenc[:], lensq[:], AF.Sqrt)
                    nc.vector.tensor_scalar_max(lenc[:], lenc[:], 1e-8)
                    nc.vector.reciprocal(invl[:], lenc[:])
                    nc.vector.tensor_scalar_mul(y1[:], vec[:], invl[:, :1])
                    # ---- radial coefficients c[e, R] ----
                    # one PSUM bank per tile iteration, sub-viewed
                    mix_ps = psp.tile([128, 512], FP, tag="mix")
                    lenT_ps = mix_ps[0:1, 0:128]
                    hpre_ps = mix_ps[0:64, 128:256]
                    c_ps = mix_ps[:, 256:256 + R]
                    nc.tensor.matmul(
                        lenT_ps, lhsT=lenc[:], rhs=ident_sb[:],
                        start=True, stop=True,
                    )
                    lenrow = geop.tile([1, 128], FP, tag="lenrow")
                    nc.scalar.activation(lenrow[:], lenT_ps, AF.Copy)
                    nc.tensor.matmul(
                        hpre_ps, lhsT=ones_sb[:1, :], rhs=lenrow[:1, :],
                        start=True, stop=True,
                    )
                    gT = geop.tile([NKNOT, 128], FP, tag="gT")
                    nc.scalar.activation(
                        gT[:], hpre_ps, AF.Relu, bias=knots_sb[:, 0:1]
                    )
                    nc.tensor.matmul(
                        c_ps, lhsT=gT[:], rhs=v_sb[:], start=True, stop=True
                    )
                    c_sb = geop.tile([128, R], FP, tag="csb")
                    nc.scalar.activation(c_sb[:], c_ps, AF.Copy)
                    # ---- features F = [xs | xv | xvy | xsY] ----
                    ft = featp.tile([128, NF], BF, tag="F")
                    pvy = featp.tile([128, 48], FP, tag="pvy")
                    nc.vector.tensor_copy(ft[:, 0:96], xt[:, 0:96])
                    nc.vector.tensor_tensor(
                        out=pvy[:].rearrange("p (i m) -> p i m", m=3),
                        in0=xt[:, 48:96].rearrange("p (i m) -> p i m", m=3),
                        in1=y1[:].rearrange("p (o m) -> p o m", o=1).to_broadcast(
                            [128, 16, 3]
                        ),
                        op=OP.mult,
                    )
                    with nc.allow_low_precision(reason="3-term dot, bf16 out"):
                        nc.vector.reduce_sum(
                            ft[:, 96:112],
                            pvy[:].rearrange("p (i m) -> p i m", m=3),
                            axis=mybir.AxisListType.X,
                        )
                    for m in range(3):
                        nc.vector.tensor_scalar_mul(
                            ft[:, 112 + 48 * m:160 + 48 * m],
                            xt[:, 0:48],
                            y1[:, m:m + 1],
                        )
                    # ---- rank-1 expansion P[e, (r,f)] ----
                    pt = pbufp.tile([128, R * NF], BF, tag="P")
                    for r in range(R):
                        nc.vector.tensor_scalar_mul(
                            pt[:, NF * r:NF * (r + 1)], ft[:], c_sb[:, r:r + 1]
                        )
                    # ---- one-hot + fused TP/scatter ----
                    oh = featp.tile([128, 128], BF, tag="oh")
                    nc.vector.tensor_scalar(
                        out=oh[:], in0=iota_sb[:], scalar1=dlc[:, :1],
                        scalar2=None, op0=OP.is_equal,
                    )
                    for k in range(NCHUNK):
                        nc.tensor.matmul(
                            q_ps[k // 4][:, 128 * (k % 4):128 * (k % 4 + 1)],
                            lhsT=pt[:, 128 * k:128 * (k + 1)],
                            rhs=oh[:],
                            start=False,
                            stop=(t == T - 1 and (k % 4) == 3),
                        )
                # ---- bucket epilogue: weight contraction + node stage ----
                qsb = epip.tile([128, NCHUNK * 128], BF, tag="qsb")
                for j in range(3):
                    nc.scalar.activation(
                        qsb[:, 512 * j:512 * (j + 1)], q_ps[j][:], AF.Copy
                    )
                big_ps = pse.tile([128, 512], FP, tag="big")
                outT_ps = big_ps[:, 0:128]
                sT_ps = big_ps[0:48, 128:256]
                gT_ps = big_ps[0:48, 256:384]
                nsT_ps = big_ps[0:48, 384:512]
                for k in range(NCHUNK):
                    nc.tensor.matmul(
                        outT_ps,
                        lhsT=wbig_sb[:, 128 * k:128 * (k + 1)],
                        rhs=qsb[:, 128 * k:128 * (k + 1)],
                        start=(k == 0),
                        stop=(k == NCHUNK - 1),
                    )
                acc_s = epip.tile([48, 128], FP, tag="acc_s")
                acc_v = epip.tile([48, 128], FP, tag="acc_v")
                nc.scalar.activation(acc_s[:], outT_ps[0:48, :], AF.Copy)
                nc.scalar.activation(acc_v[:], outT_ps[64:112, :], AF.Copy)
                nc.tensor.matmul(
                    sT_ps, lhsT=ws_sb[:], rhs=acc_s[:], start=True, stop=True
                )
                nc.tensor.matmul(
                    gT_ps, lhsT=wg_sb[:], rhs=acc_s[:], start=True, stop=True
                )
                nc.tensor.matmul(
                    nsT_ps, lhsT=wns_sb[:], rhs=acc_v[:], start=True, stop=True
                )
                nc.scalar.activation(
                    sT_all[:, 128 * b:128 * (b + 1)], sT_ps, AF.Copy
                )
                nc.scalar.activation(
                    gT_all[:, 128 * b:128 * (b + 1)], gT_ps, AF.Copy
                )
                nc.scalar.activation(
                    ns_all[:, 128 * b:128 * (b + 1)], nsT_ps, AF.Copy
                )
            # ---- batched gated node nonlinearity (one ACT table switch/rep) --
            sig_s = nodep.tile([48, 1024], FP, tag="sig_s")
            sig_g = nodep.tile([48, 1024], FP, tag="sig_g")
            fin_all = nodep.tile([48, 1024], FP, tag="fin_all")
            nc.scalar.activation(sig_s[:], sT_all[:], AF.Sigmoid)
            nc.scalar.activation(sig_g[:], gT_all[:], AF.Sigmoid)
            nc.vector.tensor_tensor(
                out=sig_s[:], in0=sT_all[:], in1=sig_s[:], op=OP.mult
            )
            nc.vector.tensor_tensor(
                out=fin_all[:], in0=sig_g[:], in1=ns_all[:], op=OP.mult
            )
            nc.vector.tensor_tensor(
                out=fin_all[:], in0=fin_all[:], in1=sig_s[:], op=OP.add
            )
            for b in range(BUCKETS):
                finT_ps = pse.tile([128, 48], FP, tag="finT")
                nc.tensor.transpose(
                    finT_ps[:], fin_all[:, 128 * b:128 * (b + 1)],
                    ident_sb[:48, :48],
                )
                fino = epip.tile([128, 48], FP, tag="fino")
                nc.scalar.activation(fino[:], finT_ps[:], AF.Copy)
                nc.sync.dma_start(
                    out=d_out[128 * b:128 * (b + 1), :], in_=fino[:]
                )
            if rep_ctx is not None:
                rep_ctx.__exit__(None, None, None)
    nc.finalize()
    return nc


def _make_in_maps(inputs, idx_pair, dl, pdst):
    idx16, srcidx = idx_pair
    import ml_dtypes

    x = np.asarray(inputs["x"], np.float32)
    pos = np.asarray(inputs["pos"], np.float32)
    w1 = np.asarray(inputs["w1"], np.float32)
    w2 = np.asarray(inputs["w2"], np.float32)

    # node table: [x bf16 (96) | pos hi (3) | pos lo (3) | 0 pad] = 128 bf16
    xb = np.zeros((N_NODES, 128), ml_dtypes.bfloat16)
    xb[:, 0:96] = x.astype(ml_dtypes.bfloat16)
    poshi = pos.astype(ml_dtypes.bfloat16)
    poslo = (pos - poshi.astype(np.float32)).astype(ml_dtypes.bfloat16)
    xb[:, 96:99] = poshi
    xb[:, 99:102] = poslo

    # len_max from actual edge geometry (host gather, cheap)
    ei = np.asarray(inputs["edge_index"], np.int64)
    vec = pos[ei[1]] - pos[ei[0]]
    len_max = float(np.sqrt((vec * vec).sum(axis=1)).max())

    knots, a_relu, br = _radial_basis(w1, w2, len_max)
    wbig = _build_wbig(br)                          # [R*256, 128]
    wbig_packed = np.ascontiguousarray(
        wbig.reshape(NCHUNK, 128, 128).transpose(1, 0, 2).reshape(128, NCHUNK * 128)
    ).astype(ml_dtypes.bfloat16)

    ws_c = (np.asarray(inputs["Ws"], np.float32) / np.sqrt(M0)).astype(np.float32)
    wg_c = (np.asarray(inputs["Wg"], np.float32) / np.sqrt(M0)).astype(np.float32)
    wns_c = _wns_block(np.asarray(inputs["Wns"], np.float32))
    ident = np.eye(128, dtype=np.float32)
    iota = np.tile(np.arange(128, dtype=np.float32), (128, 1)).astype(
        ml_dtypes.bfloat16
    )
    in_maps = []
    for c in range(N_CORES):
        in_maps.append({
            "xb": xb,
            "idx16": np.ascontiguousarray(idx16[c]),
            "srcidx": np.ascontiguousarray(srcidx[c]),
            "dl": np.ascontiguousarray(dl[c]),
            "pdst": np.ascontiguousarray(pdst[c]),
            "wbig": wbig_packed,
            "vmat": a_relu.astype(np.float32),
            "onesrow": np.ones((1, NKNOT), np.float32),
            "nknots": (-knots.reshape(NKNOT, 1)).astype(np.float32),
            "ws": ws_c, "wg": wg_c, "wns": wns_c,
            "ident": ident, "iota": iota,
            "zrow": np.zeros((1, 512), np.float32),
        })
    return in_maps


def kernel(x, pos, edge_index, w1, w2, Ws, Wns, Wg):
    inputs = {"x": x, "pos": pos, "edge_index": edge_index, "w1": w1,
              "w2": w2, "Ws": Ws, "Wns": Wns, "Wg": Wg}
    idx16, dl, pdst, T = _prep_edges(
        np.asarray(edge_index, np.int64), np.asarray(pos, np.float32)
    )
    in_maps = _make_in_maps(inputs, idx16, dl, pdst)
    nc = build_kernel(T)
    res = run_bass_kernel_spmd(nc, in_maps, core_ids=list(range(N_CORES)))
    return np.concatenate([res.results[c]["out"] for c in range(N_CORES)], axis=0)


# revision 27
# speedup vs baseline: 3.9831x; 1.7555x over previous
"""EquivariantEdgeConv fused Bass kernel for one TRN2 chip (8 NeuronCores).

Strategy (low-rank radial weights + fused TP/scatter on the PE):
  - The per-edge tensor-product weights w(len) = silu(len*w1) @ w2 / sqrt(H)
    lie on a 1-D curve parameterized by len.  An SVD of that curve (host
    side) shows rank R=6 reproduces it to ~1e-4, so each edge only needs
    R radial coefficients c[e,r] = h(len) @ V  (V = w2 @ B_R / sqrt(H)).
  - Per edge build a 256-wide feature row
        F = [ xs(48) | xv(48) | xv.y1(16) | xs x y1 (144) ]
    and the rank-1 expansion P[e, (r,f)] = c_r * F_f  (R*256 = 1536 cols,
    bf16, formed with R tensor_scalar ops at 4x DVE mode).
  - The tensor-product contraction is FUSED INTO THE SCATTER: per 128-edge
    tile, 12 PE matmuls accumulate Q[(r,f), n] += P_chunk^T @ onehot(dst)
    into PSUM; per 128-node bucket one [1536->96] weight contraction
    (12 accumulating matmuls against the host-built W_big) yields the
    scattered messages, followed by the gated o3.Linear node stage.
  - Nodes are sharded: core c owns nodes [1024c, 1024c+1024) as 8 buckets
    of 128; edges are bucketed by destination (host side) and padded to a
    shared static tile count.  x (+ a bf16-pair encoding of pos[src]) is
    fetched with ONE dma_gather per bucket; pos[dst] / dstloc / gather
    indices are streamed as contiguous per-bucket blocks.

Self-contained: shapes hardcoded for N=8192, E=65536, irreps 48x0e+16x1o,
H=64.
"""

import sys

if "/opt/trn_rl_repo" not in sys.path:
    sys.path.insert(0, "/opt/trn_rl_repo")

import numpy as np

import concourse.bacc as bacc
import concourse.bass as bass
import concourse.mybir as mybir
import concourse.tile as tile
from concourse.bass_utils import run_bass_kernel_spmd

M0, M1, H = 48, 16, 64
N_NODES, N_EDGES, N_CORES = 8192, 65536, 8
NODES_PER_CORE = N_NODES // N_CORES          # 1024
BUCKETS = NODES_PER_CORE // 128              # 8 buckets of 128 nodes per core
R = 6                                        # radial basis rank
GATHER_MODE = __import__('os').environ.get('GATHER_MODE', 'indirect')  # gather|indirect
NF = 256                                     # per-edge feature width
NCHUNK = R * NF // 128                       # 12 P/W chunks of 128 rows
FP = mybir.dt.float32
BF = mybir.dt.bfloat16
I16 = mybir.dt.int16
I32 = mybir.dt.int32

CA = 1.0 / np.sqrt(M0 * 2.0)
CB = 1.0 / np.sqrt(3.0 * M1 * 2.0)
CC = 1.0 / np.sqrt(M0 * 2.0)
CD = 1.0 / np.sqrt(M1 * 2.0)
SQRT3 = float(np.sqrt(3.0))


def _silu64(x):
    return x / (1.0 + np.exp(-x))


NKNOT = 64


def _radial_basis(w1, w2, len_max):
    """Rank-R SVD basis of the radial weight curve w(len), evaluated on
    device through a ReLU spline: c_r(len) = relu(len - knots) @ A.

    (ReLU is used because sqrt/relu/square/copy share one ACT function
    table set on TRN2 -- no per-tile table reloads.)

    Returns knots [NKNOT], A [NKNOT, R], BR [4096, R], all float64.
    """
    grid = np.linspace(0.0, float(len_max) * 1.02 + 1e-6, 2048)
    hs = _silu64(grid[:, None] * w1.astype(np.float64)[0][None, :])   # [S,H]
    ws = hs @ w2.astype(np.float64) / np.sqrt(H)                      # [S,4096]
    _, _, vt = np.linalg.svd(ws, full_matrices=False)
    br = vt[:R].T                                                     # [4096,R]
    c_true = ws @ br                                                  # [S,R]
    knots = np.linspace(0.0, float(len_max) * 1.01, NKNOT)
    g = np.maximum(grid[:, None] - knots[None, :], 0.0)               # [S,NKNOT]
    a = np.linalg.solve(
        g.T @ g + 1e-7 * np.eye(NKNOT), g.T @ c_true
    )                                                                 # [NKNOT,R]
    return knots, a, br


def _build_wbig(br):
    """W_big [(r,f), q] mapping rank-1 features to the 96 message outputs.

    f layout: [xs(48) | xv(i,m)(48) | xvy(16) | xsY(m-major,144)]
    q layout: [ms o<48 | pad(16) | mv 64+3o+m | pad(16)]  (mv at partition
    base 64 so the epilogue ACT copy reads at a legal partition offset)
    Returns [R*256, 128] float64.
    """
    wb = np.zeros((R, NF, 128))
    # a/b/c/d carry the trailing R axis ([i, o, R])
    a = br[:2304].reshape(48, 48, R)
    b = br[2304:3072].reshape(16, 48, R)
    c = br[3072:3840].reshape(48, 16, R)
    d = br[3840:4096].reshape(16, 16, R)
    for r in range(R):
        # path A: f=i (xs), q=o
        wb[r, 0:48, 0:48] = CA * a[:, :, r]
        # path B: f=96+i (xvy), q=o  (sqrt3 from Y1)
        wb[r, 96:112, 0:48] = CB * SQRT3 * b[:, :, r]
        # path D: f=48+3i+m (xv), q=64+3o+m
        for m in range(3):
            wb[r, 48 + m:96:3, 64 + m:112:3] = CD * d[:, :, r]
            # path C: f=112+48m+i (xsY), q=64+3o+m  (sqrt3 from Y1)
            wb[r, 112 + 48 * m:160 + 48 * m, 64 + m:112:3] = CC * SQRT3 * c[:, :, r]
    return wb.reshape(R * NF, 128)


def _wns_block(wns):
    """[48,48] lhsT for the 1o o3.Linear on (o,m)-interleaved rows."""
    out = np.zeros((48, 48), np.float32)
    for i in range(16):
        for m in range(3):
            for o in range(16):
                out[i * 3 + m, o * 3 + m] = wns[i, o] / np.sqrt(M1)
    return out


def _prep_edges(edge_index, pos):
    """Bucket/pad edges by destination.

    Returns per-core arrays:
      idx16  [N_CORES, BUCKETS*128, T*8]  int16  (dma_gather wrapped+replicated)
      dl     [N_CORES, BUCKETS*128, T]    fp32   (local dst, 300 for padding)
      pdst   [N_CORES, BUCKETS*128, T*4]  fp32   (pos[dst], w-padded)
    and the shared tiles-per-bucket count T.
    """
    src = edge_index[0].astype(np.int64)
    dst = edge_index[1].astype(np.int64)
    gb = dst >> 7
    order = np.argsort(gb, kind="stable")
    src_s, dst_s = src[order], dst[order]
    counts = np.bincount(gb[order], minlength=64)
    cap = max(int(np.ceil(counts.max() / 128) * 128), 128)
    T = cap // 128
    starts = np.concatenate([[0], np.cumsum(counts)])

    pos = np.asarray(pos, np.float32)
    idx16 = np.zeros((N_CORES, BUCKETS * 128, T * 8), np.int16)
    srcidx = np.zeros((N_CORES, BUCKETS * 128, T), np.int32)
    dl = np.full((N_CORES, BUCKETS * 128, T), 300.0, np.float32)
    pdst = np.zeros((N_CORES, BUCKETS * 128, T * 4), np.float32)

    for g in range(64):
        ccore, b = g >> 3, g & 7
        s, e = starts[g], starts[g + 1]
        n = e - s
        sidx = np.zeros(cap, np.int64)
        sidx[:n] = src_s[s:e]
        dloc = np.full(cap, 300.0, np.float32)
        dloc[:n] = (dst_s[s:e] - (g << 7)).astype(np.float32)
        pd = np.zeros((cap, 3), np.float32)
        pd[:n] = pos[dst_s[s:e]]
        pd[n:] = pos[0]  # padding: same as pos[src=0] so vec==0, no NaNs
        # edge k -> partition k%128, tile k//128
        k = np.arange(cap)
        p, t = k % 128, k // 128
        r0 = 128 * b
        dl[ccore, r0 + p, t] = dloc
        srcidx[ccore, r0 + p, t] = sidx.astype(np.int32)
        pdst[ccore, r0 + p[:, None], 4 * t[:, None] + np.arange(3)[None, :]] = pd
        # gather idx wrap: idx k -> [k%16, k//16], replicated to 128 partitions
        wrapped = np.zeros((16, T * 8), np.int16)
        wrapped[k % 16, k // 16] = sidx.astype(np.int16)
        idx16[ccore, r0:r0 + 128, :] = np.tile(wrapped, (8, 1))
    return (idx16, srcidx), dl, pdst, T


def build_kernel(tiles_per_bucket: int, reps: int = 1) -> bass.Bass:
    T = tiles_per_bucket
    assert T <= 10, "radial PSUM layout sized for T<=10"
    OHW = R * 128                         # scaled-onehot width (768)
    RHS_PARTS = [(0, min(512, OHW))] + ([(512, OHW)] if OHW > 512 else [])
    nc = bacc.Bacc(None, target_bir_lowering=False, debug=False)
    d_xb = nc.declare_dram_parameter("xb", [N_NODES, 128], BF, isOutput=False)
    d_idx = nc.declare_dram_parameter("idx16", [BUCKETS * 128, T * 8], I16, isOutput=False)
    d_srcidx = nc.declare_dram_parameter("srcidx", [BUCKETS * 128, T], I32, isOutput=False)
    d_dl = nc.declare_dram_parameter("dl", [BUCKETS * 128, T], FP, isOutput=False)
    d_pd = nc.declare_dram_parameter("pdst", [BUCKETS * 128, T * 4], FP, isOutput=False)
    d_wbig = nc.declare_dram_parameter("wbig", [128, NCHUNK * 128], FP, isOutput=False)
    d_v = nc.declare_dram_parameter("vmat", [NKNOT, R], FP, isOutput=False)
    d_knots = nc.declare_dram_parameter("nknots", [NKNOT, 1], FP, isOutput=False)
    d_ws = nc.declare_dram_parameter("ws", [M0, M0], FP, isOutput=False)
    d_wg = nc.declare_dram_parameter("wg", [M0, M0], FP, isOutput=False)
    d_wns = nc.declare_dram_parameter("wns", [48, 48], FP, isOutput=False)
    d_ident = nc.declare_dram_parameter("ident", [128, 128], FP, isOutput=False)
    d_iota = nc.declare_dram_parameter("iota", [128, 128], BF, isOutput=False)
    d_out = nc.declare_dram_parameter("out", [NODES_PER_CORE, M0], FP, isOutput=True)

    AF = mybir.ActivationFunctionType
    OP = mybir.AluOpType

    with tile.TileContext(nc) as tc, tc.tile_pool(name="consts", bufs=1) as cp:
        wbig_sb = cp.tile([128, NCHUNK * 128], FP)
        v_sb = cp.tile([NKNOT, R], FP)
        knots_sb = cp.tile([NKNOT, 1], FP)
        ws_sb = cp.tile([M0, M0], FP)
        wg_sb = cp.tile([M0, M0], FP)
        wns_sb = cp.tile([48, 48], FP)
        ident_sb = cp.tile([128, 128], FP)
        iota_sb = cp.tile([128, 128], BF)
        for sb, dr in (
            (wbig_sb, d_wbig), (v_sb, d_v), (knots_sb, d_knots), (ws_sb, d_ws),
            (wg_sb, d_wg), (wns_sb, d_wns), (ident_sb, d_ident), (iota_sb, d_iota),
        ):
            nc.sync.dma_start(out=sb[:], in_=dr[:])

        with (
            tc.tile_pool(name="bkt", bufs=2) as bktp,
            tc.tile_pool(name="geo", bufs=2) as geop,
            tc.tile_pool(name="fall", bufs=2) as fallp,
            tc.tile_pool(name="ohp", bufs=3) as ohp,
            tc.tile_pool(name="epi", bufs=2) as epip,
            tc.tile_pool(name="node", bufs=1) as nodep,
            tc.tile_pool(name="qacc", bufs=1, space="PSUM") as qaccp,
            tc.tile_pool(name="rad", bufs=1, space="PSUM") as radp,
            tc.tile_pool(name="ps_epi", bufs=1, space="PSUM") as pse,
        ):
            rep_ctx = tc.For_i(0, reps, 1) if reps > 1 else None
            if rep_ctx is not None:
                rep_ctx.__enter__()
            sT_all = nodep.tile([48, 1024], FP, tag="sT_all")
            gT_all = nodep.tile([48, 1024], FP, tag="gT_all")
            ns_all = nodep.tile([48, 1024], FP, tag="ns_all")
            fino_bufs = []
            for b in range(BUCKETS):
                dlb = bktp.tile([128, T], FP, tag="dl")
                pdb = bktp.tile([128, T * 4], FP, tag="pd")
                xgb = bktp.tile([128, T * 128], BF, tag="xgb")
                r0 = 128 * b
                nc.sync.dma_start(out=dlb[:], in_=d_dl[r0:r0 + 128, :])
                nc.sync.dma_start(out=pdb[:], in_=d_pd[r0:r0 + 128, :])
                if GATHER_MODE == "gather":
                    idxt = bktp.tile([128, T * 8], I16, tag="idx")
                    nc.sync.dma_start(out=idxt[:], in_=d_idx[r0:r0 + 128, :])
                    nc.gpsimd.dma_gather(
                        out_ap=xgb[:].rearrange("p (t e) -> p t e", e=128),
                        in_ap=d_xb[:, :],
                        idxs_ap=idxt[:],
                        num_idxs=T * 128,
                        num_idxs_reg=T * 128,
                        elem_size=128,
                        single_packet=False,
                    )
                else:
                    sidxt = bktp.tile([128, T], I32, tag="sidx")
                    nc.sync.dma_start(out=sidxt[:], in_=d_srcidx[r0:r0 + 128, :])
                    for tt in range(T):
                        nc.gpsimd.indirect_dma_start(
                            out=xgb[:, 128 * tt:128 * (tt + 1)],
                            out_offset=None,
                            in_=d_xb[:],
                            in_offset=bass.IndirectOffsetOnAxis(
                                ap=sidxt[:, tt:tt + 1], axis=0
                            ),
                        )
                # ---- batched edge geometry (whole bucket at once) ----
                vec_all = geop.tile([128, T * 3], FP, tag="vec")
                sq_all = geop.tile([128, T * 3], FP, tag="sq")
                lensq = geop.tile([128, T], FP, tag="lensq")
                len_all = geop.tile([128, T], FP, tag="len")
                invl = geop.tile([128, T], FP, tag="invl")
                y1_all = geop.tile([128, T * 3], FP, tag="y1")
                xgb_t3 = xgb[:].rearrange("p (t e) -> p t e", e=128)
                nc.vector.tensor_tensor(
                    out=vec_all[:].rearrange("p (t m) -> p t m", m=3),
                    in0=pdb[:].rearrange("p (t m) -> p t m", m=4)[:, :, 0:3],
                    in1=xgb_t3[:, :, 96:99],
                    op=OP.subtract,
                )
                nc.vector.tensor_tensor(
                    out=vec_all[:].rearrange("p (t m) -> p t m", m=3),
                    in0=vec_all[:].rearrange("p (t m) -> p t m", m=3),
                    in1=xgb_t3[:, :, 99:102],
                    op=OP.subtract,
                )
                nc.vector.tensor_tensor(
                    out=sq_all[:].rearrange("p (t m) -> p t m", m=3),
                    in0=vec_all[:].rearrange("p (t m) -> p t m", m=3),
                    in1=vec_all[:].rearrange("p (t m) -> p t m", m=3),
                    op=OP.mult,
                )
                nc.vector.reduce_sum(
                    lensq[:], sq_all[:].rearrange("p (t m) -> p t m", m=3),
                    axis=mybir.AxisListType.X,
                )
                nc.scalar.activation(len_all[:], lensq[:], AF.Sqrt)
                nc.vector.tensor_scalar_max(len_all[:], len_all[:], 1e-8)
                nc.vector.reciprocal(invl[:], len_all[:])
                nc.vector.tensor_tensor(
                    out=y1_all[:].rearrange("p (t m) -> p t m", m=3),
                    in0=vec_all[:].rearrange("p (t m) -> p t m", m=3),
                    in1=invl[:].rearrange("p (t m) -> p t m", m=1).to_broadcast(
                        [128, T, 3]
                    ),
                    op=OP.mult,
                )
                # ---- radial coefficients (bucket-batched relu spline) ----
                rad_ps = radp.tile([128, 1536], FP, tag="rad")
                gpre = rad_ps[0:NKNOT, 0:T * 128]
                c_ps = rad_ps[:, T * 128:T * 128 + R * T]
                for t in range(T):
                    # gpre[k, e] = len[e]: transpose-trick matmul with a
                    # broadcast (stride-0) stationary operand
                    nc.tensor.matmul(
                        gpre[:, 128 * t:128 * (t + 1)],
                        lhsT=len_all[:, t:t + 1].to_broadcast([128, NKNOT]),
                        rhs=ident_sb[:],
                        start=True, stop=True,
                    )
                g_sb = geop.tile([NKNOT, T * 128], FP, tag="g_sb")
                nc.scalar.activation(
                    g_sb[:], gpre, AF.Relu, bias=knots_sb[:, 0:1]
                )
                for t in range(T):
                    nc.tensor.matmul(
                        c_ps[:, R * t:R * (t + 1)],
                        lhsT=g_sb[:, 128 * t:128 * (t + 1)],
                        rhs=v_sb[:],
                        start=True, stop=True,
                    )
                c_sb = geop.tile([128, R * T], FP, tag="c_sb")
                nc.scalar.activation(c_sb[:], c_ps, AF.Copy)
                # ---- batched features F_all = [xs | xv | xvy | xsY] ----
                f_all = fallp.tile([128, T * NF], BF, tag="F")
                f_t = f_all[:].rearrange("p (t f) -> p t f", f=NF)
                pvy = fallp.tile([128, T * 48], FP, tag="pvy")
                nc.vector.tensor_copy(f_t[:, :, 0:96], xgb_t3[:, :, 0:96])
                nc.vector.tensor_tensor(
                    out=pvy[:].rearrange("p (t i m) -> p t i m", i=16, m=3),
                    in0=xgb_t3[:, :, 48:96].rearrange(
                        "p t (i m) -> p t i m", m=3
                    ),
                    in1=y1_all[:].rearrange("p (t o m) -> p t o m", o=1, m=3)
                    .to_broadcast([128, T, 16, 3]),
                    op=OP.mult,
                )
                with nc.allow_low_precision(reason="3-term dot, bf16 out"):
                    nc.vector.reduce_sum(
                        f_t[:, :, 96:112],
                        pvy[:].rearrange("p (t i m) -> p t i m", i=16, m=3),
                        axis=mybir.AxisListType.X,
                    )
                nc.vector.tensor_tensor(
                    out=f_t[:, :, 112:256].rearrange(
                        "p t (m i) -> p t m i", i=48
                    ),
                    in0=xgb_t3[:, :, 0:48].rearrange(
                        "p t (o i) -> p t o i", o=1
                    ).to_broadcast([128, T, 3, 48]),
                    in1=y1_all[:].rearrange("p (t m o) -> p t m o", m=3, o=1)
                    .to_broadcast([128, T, 3, 48]),
                    op=OP.mult,
                )
                # ---- per-tile: scaled one-hots + fused TP/scatter ----
                q_ps = qaccp.tile([128, 2048], FP, tag="q")
                for t in range(T):
                    oh_all = ohp.tile([128, OHW], BF, tag="oh")
                    for r in range(R):
                        nc.vector.tensor_scalar(
                            out=oh_all[:, 128 * r:128 * (r + 1)],
                            in0=iota_sb[:],
                            scalar1=dlb[:, t:t + 1],
                            scalar2=c_sb[:, R * t + r:R * t + r + 1],
                            op0=OP.is_equal,
                            op1=OP.mult,
                        )
                    for h in (0, 1):
                        for (c0, c1) in RHS_PARTS:
                            nc.tensor.matmul(
                                q_ps[:, 1024 * h + c0:1024 * h + c1],
                                lhsT=f_all[:, NF * t + 128 * h:
                                           NF * t + 128 * (h + 1)],
                                rhs=oh_all[:, c0:c1],
                                start=(t == 0),
                                stop=(t == T - 1),
                            )
                # ---- bucket epilogue: weight contraction + node linears ----
                qsb = epip.tile([128, NCHUNK * 128], FP, tag="qsb")
                nc.scalar.activation(qsb[:, 0:OHW], q_ps[:, 0:OHW], AF.Copy)
                nc.scalar.activation(
                    qsb[:, OHW:2 * OHW], q_ps[:, 1024:1024 + OHW], AF.Copy
                )
                big_ps = pse.tile([128, 512], FP, tag="big")
                outT_ps = big_ps[:, 0:128]
                sT_ps = big_ps[0:48, 128:256]
                gT_ps = big_ps[0:48, 256:384]
                nsT_ps = big_ps[0:48, 384:512]
                for k in range(NCHUNK):
                    r, h = k // 2, k % 2
                    nc.tensor.matmul(
                        outT_ps,
                        lhsT=wbig_sb[:, 128 * k:128 * (k + 1)],
                        rhs=qsb[:, OHW * h + 128 * r:OHW * h + 128 * (r + 1)],
                        start=(k == 0),
                        stop=(k == NCHUNK - 1),
                    )
                acc_s = epip.tile([48, 128], FP, tag="acc_s")
                acc_v = epip.tile([48, 128], FP, tag="acc_v")
                nc.scalar.activation(acc_s[:], outT_ps[0:48, :], AF.Copy)
                nc.scalar.activation(acc_v[:], outT_ps[64:112, :], AF.Copy)
                nc.tensor.matmul(
                    sT_ps, lhsT=ws_sb[:], rhs=acc_s[:], start=True, stop=True
                )
                nc.tensor.matmul(
                    gT_ps, lhsT=wg_sb[:], rhs=acc_s[:], start=True, stop=True
                )
                nc.tensor.matmul(
                    nsT_ps, lhsT=wns_sb[:], rhs=acc_v[:], start=True, stop=True
                )
                nc.scalar.activation(
                    sT_all[:, 128 * b:128 * (b + 1)], sT_ps, AF.Copy
                )
                nc.scalar.activation(
                    gT_all[:, 128 * b:128 * (b + 1)], gT_ps, AF.Copy
                )
                nc.scalar.activation(
                    ns_all[:, 128 * b:128 * (b + 1)], nsT_ps, AF.Copy
                )
            # ---- batched gated node nonlinearity (one ACT table switch/rep) --
            sig_s = nodep.tile([48, 1024], FP, tag="sig_s")
            sig_g = nodep.tile([48, 1024], FP, tag="sig_g")
            fin_all = nodep.tile([48, 1024], FP, tag="fin_all")
            nc.scalar.activation(sig_s[:], sT_all[:], AF.Sigmoid)
            nc.scalar.activation(sig_g[:], gT_all[:], AF.Sigmoid)
            nc.vector.tensor_tensor(
                out=sig_s[:], in0=sT_all[:], in1=sig_s[:], op=OP.mult
            )
            nc.vector.tensor_tensor(
                out=fin_all[:], in0=sig_g[:], in1=ns_all[:], op=OP.mult
            )
            nc.vector.tensor_tensor(
                out=fin_all[:], in0=fin_all[:], in1=sig_s[:], op=OP.add
            )
            for b in range(BUCKETS):
                big_ps = pse.tile([128, 512], FP, tag="big")
                finT_ps = big_ps[:, 48 * (b % 2):48 * (b % 2) + 48]
                nc.tensor.transpose(
                    finT_ps, fin_all[:, 128 * b:128 * (b + 1)],
                    ident_sb[:48, :48],
                )
                fino = epip.tile([128, 48], FP, tag="fino")
                nc.scalar.activation(fino[:], finT_ps, AF.Copy)
                nc.sync.dma_start(
                    out=d_out[128 * b:128 * (b + 1), :], in_=fino[:]
                )
            if rep_ctx is not None:
                rep_ctx.__exit__(None, None, None)
    nc.finalize()
    return nc


def _make_in_maps(inputs, idx_pair, dl, pdst):
    idx16, srcidx = idx_pair
    import ml_dtypes

    x = np.asarray(inputs["x"], np.float32)
    pos = np.asarray(inputs["pos"], np.float32)
    w1 = np.asarray(inputs["w1"], np.float32)
    w2 = np.asarray(inputs["w2"], np.float32)

    # node table: [x bf16 (96) | pos hi (3) | pos lo (3) | 0 pad] = 128 bf16
    xb = np.zeros((N_NODES, 128), ml_dtypes.bfloat16)
    xb[:, 0:96] = x.astype(ml_dtypes.bfloat16)
    poshi = pos.astype(ml_dtypes.bfloat16)
    poslo = (pos - poshi.astype(np.float32)).astype(ml_dtypes.bfloat16)
    xb[:, 96:99] = poshi
    xb[:, 99:102] = poslo

    # len_max from actual edge geometry (host gather, cheap)
    ei = np.asarray(inputs["edge_index"], np.int64)
    vec = pos[ei[1]] - pos[ei[0]]
    len_max = float(np.sqrt((vec * vec).sum(axis=1)).max())

    knots, a_relu, br = _radial_basis(w1, w2, len_max)
    wbig = _build_wbig(br)                          # [R*256, 128]
    wbig_packed = np.ascontiguousarray(
        wbig.reshape(NCHUNK, 128, 128).transpose(1, 0, 2).reshape(128, NCHUNK * 128)
    ).astype(np.float32)

    ws_c = (np.asarray(inputs["Ws"], np.float32) / np.sqrt(M0)).astype(np.float32)
    wg_c = (np.asarray(inputs["Wg"], np.float32) / np.sqrt(M0)).astype(np.float32)
    wns_c = _wns_block(np.asarray(inputs["Wns"], np.float32))
    ident = np.eye(128, dtype=np.float32)
    iota = np.tile(np.arange(128, dtype=np.float32), (128, 1)).astype(
        ml_dtypes.bfloat16
    )
    in_maps = []
    for c in range(N_CORES):
        in_maps.append({
            "xb": xb,
            "idx16": np.ascontiguousarray(idx16[c]),
            "srcidx": np.ascontiguousarray(srcidx[c]),
            "dl": np.ascontiguousarray(dl[c]),
            "pdst": np.ascontiguousarray(pdst[c]),
            "wbig": wbig_packed,
            "vmat": a_relu.astype(np.float32),
            "nknots": (-knots.reshape(NKNOT, 1)).astype(np.float32),
            "ws": ws_c, "wg": wg_c, "wns": wns_c,
            "ident": ident, "iota": iota,
        })
    return in_maps


def kernel(x, pos, edge_index, w1, w2, Ws, Wns, Wg):
    inputs = {"x": x, "pos": pos, "edge_index": edge_index, "w1": w1,
              "w2": w2, "Ws": Ws, "Wns": Wns, "Wg": Wg}
    idx16, dl, pdst, T = _prep_edges(
        np.asarray(edge_index, np.int64), np.asarray(pos, np.float32)
    )
    in_maps = _make_in_maps(inputs, idx16, dl, pdst)
    nc = build_kernel(T)
    res = run_bass_kernel_spmd(nc, in_maps, core_ids=list(range(N_CORES)))
    return np.concatenate([res.results[c]["out"] for c in range(N_CORES)], axis=0)
